# revision 1
# baseline (speedup 1.0000x reference)
"""Distributed multi-head attention kernel for 8 TRN2 NeuronCores.

Sharding: core c handles batch b = c//2 and head-group hg = c%2 (4 of 8
heads = 256 output columns).  Output slices are disjoint -> no collectives;
the host concatenates the 8 slices.

Device algorithm (per core), bf16 matmuls / f32 softmax+finalize:
  - host permutes the key axis (unmasked keys first, ascending) and
    transposes inputs to [D, S]; only the first NU=ceil(max_unmasked/128)
    key chunks enter scores/exp/PV (sparse attention over v_mask) -- the
    remaining masked keys would contribute exactly exp(-1e10) = 0
  - input DMAs stream in column halves split across both HWDGE queues
    (sync + scalar) to bound per-queue issue serialization
  - scores in S^T layout [k', q]; the two heads of a pair use PE row
    groups 0-63 / 64-127 so their score matmuls run concurrently; one
    [128, 1024] PSUM tile holds both heads' scores for a q-tile and a
    single ACT exp (per-partition key bias; scale=0.125) covers both
  - causal masking: block-level skips from a liveness structure computed
    from v_mask (union over batches so the SPMD graph is identical on all
    cores); straddling blocks get per-core 0/1 masks multiplied in (bf16)
  - PV: O^T[65, q] accumulated in PSUM over key chunks; row 64 (ones
    column appended to VW) is the softmax denominator
  - the 8 (head-pair dc, q-tile t) passes are emitted t-major so the two
    head-pairs' independent scores->exp->mask->PV chains interleave
    (PSUM: scores 2 slots x 2 banks, accumulators+transposes 4 banks);
    dc=1's K/Q projections are emitted just-in-time inside dc=0 passes
  - dead queries (all causally-allowed keys masked; 0/0 in exact math but
    the reference's fp32 rounding yields a uniform average over
    singly-masked keys): host precomputes fvec = v_perm^T @ F; 16 tiny
    matmuls against the resident Wv weights add the fix into output
    columns 0..3, with the dead-slot counts joined at finalize -- this
    frees the masked-tail value chunks entirely, so their projections
    and loads are skipped
  - finalize per pass: PE-transpose O^T -> [q, 65] (bf16), scale by
    q_mask/rowsum, per-q-tile output DMA
"""

import numpy as np
import ml_dtypes

BF = ml_dtypes.bfloat16
B, S, D = 4, 2048, 512
HG = 256          # output columns per core (4 heads x 64)
KS = 65           # head value width + ones column
NCH = 16          # total key chunks of 128
NEG = np.float32(-1e10)

_CACHE = {}


def _structure(v_mask):
    """Key permutations + block liveness (union over batches -> SPMD-safe)."""
    perms, n1s = [], []
    for b in range(B):
        unm = np.where(v_mask[b] == 1)[0]
        msk = np.where(v_mask[b] == 0)[0]
        perms.append(np.concatenate([unm, msk]))
        n1s.append(len(unm))
    NU = int(max(-(-n // 128) for n in n1s))
    live = set()
    band = set()
    for b in range(B):
        unm = perms[b][:n1s[b]]
        for c in range(NU):
            seg = unm[128 * c:min(128 * (c + 1), n1s[b])]
            if len(seg) == 0:
                continue
            lo, hi = int(seg[0]), int(seg[-1])
            for t in range(4):
                if lo > 512 * t + 511:
                    continue
                live.add((c, t))
                if hi > 512 * t:
                    band.add((c, t))
    live_lists = tuple(tuple(sorted(c for (c, tt) in live if tt == t))
                       for t in range(4))
    band_list = tuple(sorted(band))
    return perms, n1s, NU, live_lists, band_list


def _build(NU, live_lists, band_list):
    import concourse.bass as bass  # noqa: F401
    from concourse import bacc
    import concourse.mybir as mybir
    from concourse.tile import TileContext

    F32 = mybir.dt.float32
    BF16 = mybir.dt.bfloat16
    Exp = mybir.ActivationFunctionType.Exp
    nband = len(band_list)
    band_idx = {ct: i for i, ct in enumerate(band_list)}
    kp_tiles = -(-NU * 128 // 512)  # s-tiles of K to project

    nc = bacc.Bacc()
    qT = nc.declare_dram_parameter("qT", [D, S], BF16, isOutput=False)
    kT = nc.declare_dram_parameter("kT", [D, S], BF16, isOutput=False)
    vT = nc.declare_dram_parameter("vT", [D, S], BF16, isOutput=False)
    wq = nc.declare_dram_parameter("wq", [D, HG], BF16, isOutput=False)
    wk = nc.declare_dram_parameter("wk", [D, HG], BF16, isOutput=False)
    wv = nc.declare_dram_parameter("wv", [D, HG], BF16, isOutput=False)
    vbias = nc.declare_dram_parameter("vbias", [128, NCH], F32, isOutput=False)
    qmask = nc.declare_dram_parameter("qmask", [128, NCH], F32, isOutput=False)
    bmask = nc.declare_dram_parameter("bmask", [128, nband * 512], BF16,
                                      isOutput=False)
    fvec = nc.declare_dram_parameter("fvec", [128, 16], BF16, isOutput=False)
    cnt = nc.declare_dram_parameter("cnt", [128, 4], F32, isOutput=False)
    ident = nc.declare_dram_parameter("ident", [128, 128], BF16, isOutput=False)
    ones4 = nc.declare_dram_parameter("ones4", [128, 4], BF16, isOutput=False)
    out = nc.declare_dram_parameter("out", [S, HG], F32, isOutput=True)

    with TileContext(nc) as tc:
        with tc.tile_pool(name="sb", bufs=1) as sb, \
             tc.tile_pool(name="ps", bufs=1, space="PSUM") as ps:

            def sbt(name, shape, dtype, bufs=1, tag=None):
                return sb.tile(shape, dtype, name=name, tag=tag or name, bufs=bufs)

            # input tiles first; loads stream in column halves on both HWDGE queues
            def decl_xT(pfx):
                return [sb.tile([128, S], BF16, name=f"{pfx}xT{Dc}",
                                tag=f"{pfx}xT{Dc}", bufs=1) for Dc in range(4)]

            vt = decl_xT("v")
            kt = decl_xT("k")
            qt = decl_xT("q")

            def load_half(tiles, dram, hf):
                for Dc in range(4):
                    eng = nc.sync if Dc % 2 == 0 else nc.scalar
                    eng.dma_start(
                        out=tiles[Dc][:, 1024 * hf:1024 * (hf + 1)],
                        in_=dram[128 * Dc:128 * (Dc + 1), 1024 * hf:1024 * (hf + 1)])

            w_sb = {}

            def load_w(nm, dram):
                for Dc in range(4):
                    t = sbt(f"w{nm}{Dc}", [128, HG], BF16)
                    eng = nc.sync if Dc % 2 == 0 else nc.scalar
                    eng.dma_start(out=t, in_=dram[128 * Dc:128 * (Dc + 1), :])
                    w_sb[(nm, Dc)] = t

            load_half(vt, vT, 0)
            load_w("v", wv)
            load_half(kt, kT, 0)
            load_half(qt, qT, 0)
            load_w("k", wk)
            load_w("q", wq)
            klim = NU * 128
            if klim > 1024:
                for Dc in range(4):
                    eng = nc.sync if Dc % 2 == 0 else nc.scalar
                    eng.dma_start(out=vt[Dc][:, 1024:klim],
                                  in_=vT[128 * Dc:128 * (Dc + 1), 1024:klim])
            if klim > 1024:
                for Dc in range(4):
                    eng = nc.sync if Dc % 2 == 0 else nc.scalar
                    eng.dma_start(out=kt[Dc][:, 1024:klim],
                                  in_=kT[128 * Dc:128 * (Dc + 1), 1024:klim])
            load_half(qt, qT, 1)

            vbias_sb = sbt("vbias_sb", [128, NCH], F32)
            nc.sync.dma_start(out=vbias_sb, in_=vbias[:])
            qmask_sb = sbt("qmask_sb", [128, NCH], F32)
            nc.scalar.dma_start(out=qmask_sb, in_=qmask[:])
            bmask_sb = sbt("bmask_sb", [128, nband * 512], BF16)
            nc.sync.dma_start(out=bmask_sb, in_=bmask[:])
            fvec_sb = sbt("fvec_sb", [128, 16], BF16)
            nc.scalar.dma_start(out=fvec_sb, in_=fvec[:])
            cnt_sb = sbt("cnt_sb", [128, 4], F32)
            nc.sync.dma_start(out=cnt_sb, in_=cnt[:])
            ident_sb = sbt("ident_sb", [128, 128], BF16)
            nc.sync.dma_start(out=ident_sb, in_=ident[:])
            ones4_sb = sbt("ones4_sb", [128, 4], BF16)
            nc.scalar.dma_start(out=ones4_sb, in_=ones4[:])

            qwT = [sbt(f"qwT{i}", [128, S], BF16) for i in range(2)]
            kwT = [sbt(f"kwT{i}", [128, S], BF16) for i in range(2)]
            vw = [sbt(f"vw{i}", [128, 4 * KS], BF16) for i in range(NU)]

            for st in range(NU):
                p = ps.tile([128, HG], F32, name="pprj", tag="psS", bufs=2)
                for Dc in range(4):
                    nc.tensor.matmul(p, vt[Dc][:, 128 * st:128 * (st + 1)],
                                     w_sb[("v", Dc)], start=(Dc == 0), stop=(Dc == 3))
                t = vw[st]
                nc.vector.tensor_copy(
                    t.rearrange("p (h j) -> p h j", j=KS)[:, :, 64:65],
                    ones4_sb.rearrange("p (h o) -> p h o", o=1))
                nc.vector.tensor_copy(
                    t.rearrange("p (h j) -> p h j", j=KS)[:, :, 0:64],
                    p.rearrange("p (h j) -> p h j", j=64))

            def proj_kq(dc, which, st2):
                xt, dst, wnm = ((kt, kwT, "k") if which == "k" else (qt, qwT, "q"))
                lim = klim if which == "k" else S
                w = min(512, lim - 512 * st2)
                p = ps.tile([128, 512], F32, name="pprj2", tag="psS", bufs=2)
                for Dc in range(4):
                    nc.tensor.matmul(
                        p[:, 0:w], w_sb[(wnm, Dc)][:, 128 * dc:128 * (dc + 1)],
                        xt[Dc][:, 512 * st2:512 * st2 + w],
                        start=(Dc == 0), stop=(Dc == 3))
                nc.vector.tensor_copy(dst[dc][:, 512 * st2:512 * st2 + w],
                                      p[:, 0:w])

            for st2 in range(kp_tiles):
                proj_kq(0, "k", st2)
            for st2 in range(4):
                proj_kq(0, "q", st2)
            # dc=1 projections emitted just-in-time during the (t, dc=0)
            # blocks; order matches what pass (t, dc=1) consumes.
            deferred = [("k", 0), ("q", 0), ("k", 1), ("q", 1),
                        ("q", 2), ("k", 2), ("q", 3)]
            deferred = [(w, s) for (w, s) in deferred
                        if s < (kp_tiles if w == "k" else 4)]

            # ---- attention: q-tile passes, dc-interleaved, compacted keys ----
            ofin = sbt("ofin", [128, NCH * HG], F32)
            for t in range(4):
                for dc in range(2):
                    h0, h1 = 2 * dc, 2 * dc + 1
                    kw_t, qw_t = kwT[dc], qwT[dc]
                    if dc == 0 and deferred:
                        for _ in range(2):
                            if deferred:
                                w_, s_ = deferred.pop(0)
                                proj_kq(1, w_, s_)
                    lc = live_lists[t]
                    psO = {}
                    for hh in (h0, h1):
                        psO[hh] = ps.tile([KS, 512], F32, name=f"psO{hh}",
                                          tag="psO", bufs=4)
                    for c in range(lc[-1] + 1):
                        if c in lc:
                            psS = ps.tile([128, 1024], F32, name="psS",
                                          tag="psS", bufs=2)
                            for i, ho in enumerate((0, 64)):
                                nc.tensor.matmul(
                                    psS[:, 512 * i:512 * (i + 1)],
                                    kw_t[ho:ho + 64, 128 * c:128 * (c + 1)],
                                    qw_t[ho:ho + 64, 512 * t:512 * (t + 1)],
                                    start=True, stop=True)
                            U = sb.tile([128, 1024], BF16, name="U", tag="U",
                                        bufs=6)
                            nc.scalar.activation(U, psS, Exp,
                                                 bias=vbias_sb[:, c:c + 1],
                                                 scale=0.125)
                            for i, hh in enumerate((h0, h1)):
                                Ui = U[:, 512 * i:512 * (i + 1)]
                                if (c, t) in band_idx:
                                    off = band_idx[(c, t)] * 512
                                    nc.vector.tensor_mul(
                                        Ui, Ui, bmask_sb[:, off:off + 512])
                                stop = (c == lc[-1]) if t > 0 else False
                                nc.tensor.matmul(psO[hh],
                                                 vw[c][:, KS * hh:KS * (hh + 1)],
                                                 Ui,
                                                 start=(c == lc[0]), stop=stop,
                                                 skip_group_check=True)
                    if t == 0:
                        # dead-query fix: psO[:, 0:4] += Wv_hh^T @ fvec
                        for hh in (h0, h1):
                            for Dc in range(4):
                                nc.tensor.matmul(
                                    psO[hh][0:64, 0:4],
                                    w_sb[("v", Dc)][:, 64 * hh:64 * (hh + 1)],
                                    fvec_sb[:, 4 * Dc:4 * (Dc + 1)],
                                    start=False, stop=(Dc == 3),
                                    skip_group_check=True)
                    # finalize this q-tile for both heads
                    for hh in (h0, h1):
                        ot = sb.tile([KS, 512], BF16, name="ot", tag="ot", bufs=2)
                        nc.vector.tensor_copy(ot, psO[hh])
                        tp = ps.tile([128, 4 * 66], BF16, name="tp", tag="psO",
                                     bufs=4)
                        for j in range(4):
                            nc.tensor.matmul(tp[:, 66 * j:66 * j + KS],
                                             ot[:, 128 * j:128 * (j + 1)],
                                             ident_sb[0:KS, 0:KS],
                                             is_transpose=True,
                                             start=(j == 0), stop=(j == 3),
                                             skip_group_check=True)
                        rs = sb.tile([128, 4], F32, name="rs", tag="rs", bufs=2)
                        if t == 0:
                            nc.vector.tensor_add(
                                rs.rearrange("p (j o) -> p j o", o=1),
                                tp.rearrange("p (j f) -> p j f", f=66)[:, :, 64:65],
                                cnt_sb.rearrange("p (j o) -> p j o", o=1))
                        else:
                            nc.vector.tensor_scalar_add(
                                rs.rearrange("p (j o) -> p j o", o=1),
                                tp.rearrange("p (j f) -> p j f", f=66)[:, :, 64:65],
                                1e-30)
                        rcp = sb.tile([128, 4], F32, name="rcp", tag="rcp", bufs=2)
                        nc.vector.reciprocal(rcp, rs)
                        scl = sb.tile([128, 4], F32, name="scl", tag="scl", bufs=2)
                        nc.vector.tensor_mul(scl, rcp, qmask_sb[:, 4 * t:4 * (t + 1)])
                        for j in range(4):
                            col = (4 * t + j) * HG + 64 * hh
                            nc.vector.tensor_scalar_mul(
                                ofin[:, col:col + 64], tp[:, 66 * j:66 * j + 64],
                                scl[:, j:j + 1])
                    if dc == 1:
                        nc.sync.dma_start(
                            out=out.rearrange("(j p) n -> p j n", p=128)[:, 4 * t:4 * (t + 1)],
                            in_=ofin.rearrange("p (j n) -> p j n", n=HG)[:, 4 * t:4 * (t + 1)])

    nc.compile()
    return nc


def _prep_inputs(q, k, v, v_mask, q_mask, Wq, Wk, Wv, perms, n1s, band_list):
    q = np.asarray(q, np.float32)
    k = np.asarray(k, np.float32)
    v = np.asarray(v, np.float32)
    v_mask = np.asarray(v_mask, np.float32)
    q_mask = np.asarray(q_mask, np.float32)
    Wq = np.asarray(Wq, np.float32)
    Wk = np.asarray(Wk, np.float32)
    Wv = np.asarray(Wv, np.float32)
    ident = np.eye(128, dtype=np.float32)
    nband = len(band_list)

    in_maps = []
    for core in range(8):
        b, hg = core // 2, core % 2
        cs = slice(hg * HG, (hg + 1) * HG)
        perm, n1 = perms[b], n1s[b]
        vb = np.where(np.arange(S) < n1, np.float32(0), NEG).astype(np.float32)
        fix = np.zeros((S, 4), np.float32)
        if v_mask[b, 0] == 0:
            first_one = int(np.argmax(v_mask[b] > 0))
            ks_ = np.arange(S)
            for dj in range(min(first_one, 4)):
                sel = ((ks_ <= dj) & (v_mask[b] == 0)) | \
                      ((ks_ > dj) & (v_mask[b] == 1))
                fix[:, dj] = sel[perm].astype(np.float32)
        fvec = (v[b][perm].T @ fix).astype(np.float32)
        cnt = np.full((128, 4), np.float32(1e-30))
        cnt[0:4, 0] += fix.sum(axis=0)
        bm = np.zeros((128, nband * 512), np.float32)
        for i, (c, t) in enumerate(band_list):
            kpos = perm[128 * c:128 * (c + 1)][:, None]
            bm[:, 512 * i:512 * (i + 1)] = (
                kpos <= (512 * t + np.arange(512))[None, :]).astype(np.float32)
        in_maps.append({
            "qT": np.ascontiguousarray(q[b].T).astype(BF),
            "kT": np.ascontiguousarray(k[b][perm].T).astype(BF),
            "vT": np.ascontiguousarray(v[b][perm].T).astype(BF),
            "wq": np.ascontiguousarray(Wq[:, cs]).astype(BF),
            "wk": np.ascontiguousarray(Wk[:, cs]).astype(BF),
            "wv": np.ascontiguousarray(Wv[:, cs]).astype(BF),
            "vbias": np.ascontiguousarray(vb.reshape(NCH, 128).T),
            "qmask": np.ascontiguousarray(q_mask[b].reshape(NCH, 128).T),
            "bmask": bm.astype(BF),
            "fvec": np.ascontiguousarray(
                fvec.reshape(4, 128, 4).transpose(1, 0, 2)
                .reshape(128, 16)).astype(BF),
            "cnt": cnt,
            "ident": ident.astype(BF),
            "ones4": np.ones((128, 4), BF),
        })
    return in_maps


def kernel(q, k, v, v_mask, q_mask, Wq, Wk, Wv, _trace=False):
    from concourse.bass_utils import run_bass_kernel_spmd

    v_mask_f = np.asarray(v_mask, np.float32)
    perms, n1s, NU, live_lists, band_list = _structure(v_mask_f)
    key = (NU, live_lists, band_list)
    if _CACHE.get("key") != key:
        _CACHE["nc"] = _build(NU, live_lists, band_list)
        _CACHE["key"] = key
    nc = _CACHE["nc"]
    in_maps = _prep_inputs(q, k, v, v_mask, q_mask, Wq, Wk, Wv,
                           perms, n1s, band_list)
    res = run_bass_kernel_spmd(nc, in_maps, core_ids=list(range(8)), trace=_trace)
    _CACHE["last_result"] = res
    full = np.zeros((B, S, 2 * HG), np.float32)
    for core in range(8):
        b, hg = core // 2, core % 2
        full[b, :, hg * HG:(hg + 1) * HG] = res.results[core]["out"]
    return full



# revision 5
# speedup vs baseline: 1.6570x; 1.6570x over previous
"""Distributed multi-head attention kernel for 8 TRN2 NeuronCores.

Sharding: core c handles batch b = c//2 and head-group hg = c%2 (4 of 8
heads = 256 output columns).  Output slices are disjoint -> no collectives.

Device algorithm (per core), bf16 matmuls / f32 softmax:
  - host compacts BOTH axes: keys with v_mask=1 first (ascending) -> only
    NU=ceil(max_unmasked_k/128) key chunks, and queries with q_mask=1
    first (ascending) -> only NQ=ceil(max_unmasked_q/512) query tiles
    (masked queries' outputs are exactly zero -> never computed)
  - scores in S^T layout [k', q']; block (c,t) computed only if causally
    live (union over batches -> SPMD-identical graph), and narrowed to
    the live q-column range [js, 512)
  - exp via scalar ACT (bias = per-key -1e10 padding mask, scale 0.125),
    writing U bf16; straddling blocks multiply a causal 0/1 mask that is
    generated ON DEVICE via tensor_scalar(is_ge, qpos, kpos)
  - PV accumulates O^T[65*4, q'] in one PSUM tile per q-tile; row 64 of
    each head (ones column in VW) is the softmax denominator
  - dead live-queries (all causal keys masked): fvec fix matmuls as in
    the reference semantics; denominator counts added on host
  - finalize is HOST-side: device only copies PSUM->SBUF (bf16) and DMAs
    raw O^T out; host divides by the denominator row, transposes, and
    scatters into unmasked query rows
  - scalar engine queue carries ONLY the exp ACTs (the critical path);
    DMA issues go to sync HWDGE / early-scalar / gpsimd SWDGE rings
  - emission is software-pipelined (PV lags its block by one) so the ACT
    queue never waits on PSUM score slots
"""

import numpy as np
import ml_dtypes

BF = ml_dtypes.bfloat16
B, S, D = 4, 2048, 512
HG = 256          # output columns per core (4 heads x 64)
KS = 65           # head value width + ones column
NEG = np.float32(-1e10)

_CACHE = {}


def _structure(v_mask, q_mask):
    """Both-axis compaction + union block liveness (SPMD-safe)."""
    kperms, kn1s, qperms, qn1s = [], [], [], []
    for b in range(B):
        unm = np.where(v_mask[b] == 1)[0]
        msk = np.where(v_mask[b] == 0)[0]
        kperms.append(np.concatenate([unm, msk]))
        kn1s.append(len(unm))
        unq = np.where(q_mask[b] == 1)[0]
        msq = np.where(q_mask[b] == 0)[0]
        qperms.append(np.concatenate([unq, msq]))
        qn1s.append(len(unq))
    NU = int(max(-(-n // 128) for n in kn1s))
    NQ = int(max(-(-n // 512) for n in qn1s))

    blocks = []   # per t: list of (c, js, band)
    for t in range(NQ):
        bl = []
        for c in range(NU):
            live = False
            band = False
            starts = []
            per_batch = []
            for b in range(B):
                kseg = kperms[b][128 * c:min(128 * (c + 1), kn1s[b])]
                qseg = qperms[b][512 * t:min(512 * (t + 1), qn1s[b])]
                if len(kseg) == 0 or len(qseg) == 0:
                    continue
                lo, hi = int(kseg[0]), int(kseg[-1])
                if lo <= int(qseg[-1]):
                    live = True
                    jl = int(np.searchsorted(qseg, lo))
                    starts.append(jl)
                    per_batch.append((jl, hi, qseg))
                else:
                    band = True  # keys exist for b but all causally dead
            if not live:
                continue
            js = min(starts)
            for jl, hi, qseg in per_batch:
                if jl > js or hi > int(qseg[jl]):
                    band = True
            bl.append((c, int(js), bool(band)))
        assert bl and bl[0][0] == 0
        if bl[0][1] != 0:
            # widen the first block to full width so PV start=True
            # initializes every psO column (extra cols are masked by bmask)
            bl[0] = (0, 0, True)
        blocks.append(tuple(bl))

    # dead live-queries (fix): count per batch of unmasked q with all
    # causally-allowed keys masked
    nfix = 0
    for b in range(B):
        if v_mask[b, 0] == 0:
            first_one = int(np.argmax(v_mask[b] > 0))
            ndead = int(np.sum(q_mask[b, :first_one] == 1))
            nfix = max(nfix, ndead)
    NF = max(nfix, 1)  # keep graph static; zero-filled if unused

    return (kperms, kn1s, qperms, qn1s, NU, NQ, tuple(blocks), NF)


def _build(NU, NQ, blocks, NF):
    import concourse.bass as bass  # noqa: F401
    from concourse import bacc
    import concourse.mybir as mybir
    from concourse.tile import TileContext

    F32 = mybir.dt.float32
    BF16 = mybir.dt.bfloat16
    Exp = mybir.ActivationFunctionType.Exp
    GE = mybir.AluOpType.is_ge
    klim = NU * 128
    qlim = NQ * 512
    kst = [min(512, klim - 512 * i) for i in range(-(-klim // 512))]
    bands = [(c, t) for t in range(NQ) for (c, js, bd) in blocks[t] if bd]
    band_idx = {ct: i for i, ct in enumerate(bands)}
    band_js = {(c, t): js for t in range(NQ) for (c, js, bd) in blocks[t]
               if bd}

    nc = bacc.Bacc()
    kT = nc.declare_dram_parameter("kT", [D, klim], BF16, isOutput=False)
    qT = nc.declare_dram_parameter("qT", [D, qlim], BF16, isOutput=False)
    vT = nc.declare_dram_parameter("vT", [D, klim], BF16, isOutput=False)
    wq = nc.declare_dram_parameter("wq", [D, HG], BF16, isOutput=False)
    wk = nc.declare_dram_parameter("wk", [D, HG], BF16, isOutput=False)
    wv = nc.declare_dram_parameter("wv", [D, HG], BF16, isOutput=False)
    vbias = nc.declare_dram_parameter("vbias", [128, NU], F32, isOutput=False)
    kpos = nc.declare_dram_parameter("kpos", [128, NU], F32, isOutput=False)
    qpos = nc.declare_dram_parameter("qpos", [128, qlim], F32, isOutput=False)
    fvec = nc.declare_dram_parameter("fvec", [128, 4 * NF], BF16,
                                     isOutput=False)
    outT = nc.declare_dram_parameter("outT", [4 * KS, qlim], BF16,
                                     isOutput=True)

    with TileContext(nc) as tc:
        with tc.tile_pool(name="sb", bufs=1) as sb, \
             tc.tile_pool(name="ps", bufs=1, space="PSUM") as ps:

            def sbt(name, shape, dtype, bufs=1, tag=None):
                return sb.tile(shape, dtype, name=name, tag=tag or name,
                               bufs=bufs)

            kt = sbt("kt", [128, 4, klim], BF16)
            qt = sbt("qt", [128, 4, qlim], BF16)
            vt = sbt("vt", [128, 4, klim], BF16)
            wk_sb = sbt("wk_sb", [128, 4, HG], BF16)
            wq_sb = sbt("wq_sb", [128, 4, HG], BF16)
            wv_sb = sbt("wv_sb", [128, 4, HG], BF16)
            vbias_sb = sbt("vbias_sb", [128, NU], F32)
            kpos_sb = sbt("kpos_sb", [128, NU], F32)
            qpos_sb = sbt("qpos_sb", [128, qlim], F32)
            fvec_sb = sbt("fvec_sb", [128, 4 * NF], BF16)
            kwT = [sbt(f"kwT{i}", [128, klim], BF16) for i in range(2)]
            qwT = [sbt(f"qwT{i}", [128, qlim], BF16) for i in range(2)]
            vw = [sbt(f"vw{i}", [128, 4 * KS], BF16) for i in range(NU)]
            bm = [sbt(f"bm{i}", [128, 512], BF16) for i in range(len(bands))]
            oT = [sbt(f"oT{t}", [KS, 4 * 512], BF16) for t in range(NQ)]

            kTr = kT.rearrange("(c p) s -> p c s", p=128)
            qTr = qT.rearrange("(c p) s -> p c s", p=128)
            vTr = vT.rearrange("(c p) s -> p c s", p=128)

            # --- DMA issues: sync HWDGE + early scalar HWDGE + gpsimd SWDGE
            nc.sync.dma_start(out=wk_sb,
                              in_=wk.rearrange("(c p) o -> p c o", p=128))
            nc.sync.dma_start(out=kt[:, :, 0:512], in_=kTr[:, :, 0:512])
            nc.scalar.dma_start(out=qt[:, :, 0:512], in_=qTr[:, :, 0:512])
            nc.scalar.dma_start(out=wq_sb,
                                in_=wq.rearrange("(c p) o -> p c o", p=128))
            nc.sync.dma_start(out=wv_sb,
                              in_=wv.rearrange("(c p) o -> p c o", p=128))
            nc.sync.dma_start(out=vt[:, :, 0:384], in_=vTr[:, :, 0:384])
            nc.sync.dma_start(out=kt[:, :, 512:klim], in_=kTr[:, :, 512:klim])
            nc.scalar.dma_start(out=vt[:, :, 384:klim],
                                in_=vTr[:, :, 384:klim])
            if qlim > 512:
                nc.scalar.dma_start(out=qt[:, :, 512:qlim],
                                    in_=qTr[:, :, 512:qlim])
            nc.gpsimd.dma_start(out=qpos_sb, in_=qpos[:])
            nc.gpsimd.dma_start(out=kpos_sb, in_=kpos[:])
            nc.gpsimd.dma_start(out=vbias_sb, in_=vbias[:])
            nc.gpsimd.dma_start(out=fvec_sb, in_=fvec[:])

            # ones columns of VW + on-device causal masks (gpsimd, off the
            # critical engines)
            for st in range(NU):
                nc.gpsimd.memset(
                    vw[st].rearrange("p (h j) -> p h j", j=KS)[:, :, 64:65],
                    1.0)
            for i, (c, t) in enumerate(bands):
                js = band_js[(c, t)]
                nc.gpsimd.tensor_scalar(
                    bm[i][:, js:512],
                    qpos_sb[:, 512 * t + js:512 * (t + 1)],
                    kpos_sb[:, c:c + 1], None, GE)

            # --- projections (JIT-scheduled below) ---
            kq_done = set()
            v_done = set()

            def proj_kq(which, dc, st2):
                if (which, dc, st2) in kq_done:
                    return
                kq_done.add((which, dc, st2))
                xt, dst, w_sb2 = ((kt, kwT, wk_sb) if which == "k"
                                  else (qt, qwT, wq_sb))
                w = kst[st2] if which == "k" else 512
                p = ps.tile([128, 1024], F32, name="pprj", tag="ps", bufs=2)
                for Dc in range(4):
                    nc.tensor.matmul(
                        p[:, 0:w],
                        w_sb2[:, Dc, 128 * dc:128 * (dc + 1)],
                        xt[:, Dc, 512 * st2:512 * st2 + w],
                        start=(Dc == 0), stop=(Dc == 3))
                nc.vector.tensor_copy(dst[dc][:, 512 * st2:512 * st2 + w],
                                      p[:, 0:w])

            def proj_v(st):
                if st in v_done:
                    return
                v_done.add(st)
                p = ps.tile([128, 1024], F32, name="pprjv", tag="ps", bufs=2)
                for Dc in range(4):
                    nc.tensor.matmul(p[:, 0:HG],
                                     vt[:, Dc, 128 * st:128 * (st + 1)],
                                     wv_sb[:, Dc, :],
                                     start=(Dc == 0), stop=(Dc == 3))
                nc.vector.tensor_copy(
                    vw[st].rearrange("p (h j) -> p h j", j=KS)[:, :, 0:64],
                    p[:, 0:HG].rearrange("p (h j) -> p h j", j=64))

            # --- attention: q-tile passes, both head-pair chains, pipelined
            for dc in range(2):
                proj_kq("q", dc, 0)
                proj_kq("k", dc, 0)

            for t in range(NQ):
                bl = blocks[t]
                nbl = len(bl)
                psO = ps.tile([KS, 4 * 512], F32, name="psO", tag="psO",
                              bufs=1)
                psO3 = psO.rearrange("p (h w) -> p h w", w=512)
                pend = []  # delayed PVs: (c, js, dc)
                for bi in range(nbl + 1):
                    if bi < nbl:
                        c, js, bd = bl[bi]
                        w = 512 - js
                        # prefetch projections for the NEXT block / q-tile
                        if bi + 1 < nbl:
                            cn = bl[bi + 1][0]
                            for dc in range(2):
                                proj_kq("k", dc, cn // 4)
                        elif t + 1 < NQ:
                            for dc in range(2):
                                proj_kq("q", dc, t + 1)
                        Us = []
                        psSs = []
                        for dc in range(2):
                            psS = ps.tile([128, 1024], F32, name="psS",
                                          tag="ps", bufs=2)
                            psS3 = psS.rearrange("p (h w) -> p h w", w=512)
                            for hp in range(2):
                                nc.tensor.matmul(
                                    psS3[:, hp, js:512],
                                    kwT[dc][64 * hp:64 * (hp + 1),
                                            128 * c:128 * (c + 1)],
                                    qwT[dc][64 * hp:64 * (hp + 1),
                                            512 * t + js:512 * (t + 1)],
                                    start=True, stop=True)
                            psSs.append(psS3)
                        for dc in range(2):
                            U = sb.tile([128, 1024], BF16, name="U", tag="U",
                                        bufs=6)
                            U3 = U.rearrange("p (h w) -> p h w", w=512)
                            nc.scalar.activation(
                                U3[:, :, js:512], psSs[dc][:, :, js:512],
                                Exp, bias=vbias_sb[:, c:c + 1], scale=0.125)
                            Us.append(U3)
                        if bd:
                            bmv = bm[band_idx[(c, t)]]
                            for dc in range(2):
                                for hp in range(2):
                                    nc.vector.tensor_mul(
                                        Us[dc][:, hp, js:512],
                                        Us[dc][:, hp, js:512],
                                        bmv[:, js:512])
                    # delayed PVs from the previous block
                    for (pc, pjs, pdc, pU3) in pend:
                        proj_v(pc)
                        for hp in range(2):
                            h = 2 * pdc + hp
                            last = (t > 0 or NF == 0) and bi == nbl
                            nc.tensor.matmul(
                                psO3[:, h, pjs:512],
                                vw[pc][:, KS * h:KS * (h + 1)],
                                pU3[:, hp, pjs:512],
                                start=(pc == 0), stop=last,
                                skip_group_check=True)
                    pend = ([(c, js, dcx, Us[dcx]) for dcx in range(2)]
                            if bi < nbl else [])
                # dead-query fix adds into q columns 0:NF at t=0
                if t == 0 and NF > 0:
                    for h in range(4):
                        for Dc in range(4):
                            nc.tensor.matmul(
                                psO3[0:64, h, 0:NF],
                                wv_sb[:, Dc, 64 * h:64 * (h + 1)],
                                fvec_sb[:, NF * Dc:NF * (Dc + 1)],
                                start=False, stop=(Dc == 3),
                                skip_group_check=True)
                nc.vector.tensor_copy(oT[t], psO)
                nc.sync.dma_start(
                    out=outT.rearrange("(h p) q -> p h q", p=KS)[
                        :, :, 512 * t:512 * (t + 1)],
                    in_=oT[t].rearrange("p (h w) -> p h w", w=512))

    nc.compile()
    return nc


def _prep_inputs(q, k, v, v_mask, q_mask, Wq, Wk, Wv, st):
    kperms, kn1s, qperms, qn1s, NU, NQ, blocks, NF = st
    klim, qlim = NU * 128, NQ * 512
    q = np.asarray(q, np.float32)
    k = np.asarray(k, np.float32)
    v = np.asarray(v, np.float32)
    Wq = np.asarray(Wq, np.float32)
    Wk = np.asarray(Wk, np.float32)
    Wv = np.asarray(Wv, np.float32)

    in_maps = []
    fin = []  # per-core host finalize info
    for core in range(8):
        b, hg = core // 2, core % 2
        cs = slice(hg * HG, (hg + 1) * HG)
        kperm, kn1 = kperms[b], kn1s[b]
        qperm, qn1 = qperms[b], qn1s[b]
        kp = kperm[:klim]
        qp = qperm[:qlim]

        ranks = np.arange(klim)
        vb = np.where(ranks < kn1, np.float32(0), NEG).astype(np.float32)
        kposv = np.where(ranks < kn1, kp, 4096).astype(np.float32)
        qposv = np.where(np.arange(qlim) < qn1, qp, 4095).astype(np.float32)

        # dead live-query fix
        fix = np.zeros((S, NF), np.float32)
        cnt = np.zeros(NF, np.float32)
        if v_mask[b, 0] == 0:
            first_one = int(np.argmax(v_mask[b] > 0))
            ks_ = np.arange(S)
            jcol = 0
            for dj in range(first_one):
                if q_mask[b, dj] != 1:
                    continue
                sel = ((ks_ <= dj) & (v_mask[b] == 0)) | \
                      ((ks_ > dj) & (v_mask[b] == 1))
                fix[:, jcol] = sel.astype(np.float32)
                cnt[jcol] = fix[:, jcol].sum()
                jcol += 1
        fv = (v[b].T @ fix).astype(np.float32)  # [512, NF]

        in_maps.append({
            "kT": np.ascontiguousarray(k[b][kp].T).astype(BF),
            "qT": np.ascontiguousarray(q[b][qp].T).astype(BF),
            "vT": np.ascontiguousarray(v[b][kp].T).astype(BF),
            "wq": np.ascontiguousarray(Wq[:, cs]).astype(BF),
            "wk": np.ascontiguousarray(Wk[:, cs]).astype(BF),
            "wv": np.ascontiguousarray(Wv[:, cs]).astype(BF),
            "vbias": np.ascontiguousarray(vb.reshape(NU, 128).T),
            "kpos": np.ascontiguousarray(kposv.reshape(NU, 128).T),
            "qpos": np.ascontiguousarray(
                np.broadcast_to(qposv, (128, qlim))),
            "fvec": np.ascontiguousarray(
                fv.reshape(4, 128, NF).transpose(1, 0, 2)
                .reshape(128, 4 * NF)).astype(BF),
        })
        fin.append((b, hg, qp, qn1, cnt))
    return in_maps, fin


def kernel(q, k, v, v_mask, q_mask, Wq, Wk, Wv, _trace=False):
    from concourse.bass_utils import run_bass_kernel_spmd

    v_mask_f = np.asarray(v_mask, np.float32)
    q_mask_f = np.asarray(q_mask, np.float32)
    st = _structure(v_mask_f, q_mask_f)
    kperms, kn1s, qperms, qn1s, NU, NQ, blocks, NF = st
    key = (NU, NQ, blocks, NF)
    if _CACHE.get("key") != key:
        _CACHE["nc"] = _build(NU, NQ, blocks, NF)
        _CACHE["key"] = key
    nc = _CACHE["nc"]
    in_maps, fin = _prep_inputs(q, k, v, v_mask_f, q_mask_f, Wq, Wk, Wv, st)
    res = run_bass_kernel_spmd(nc, in_maps, core_ids=list(range(8)),
                               trace=_trace)
    _CACHE["last_result"] = res

    qlim = NQ * 512
    full = np.zeros((B, S, 2 * HG), np.float32)
    for core in range(8):
        b, hg, qp, qn1, cnt = fin[core]
        o = np.asarray(res.results[core]["outT"], np.float32)  # [260, qlim]
        o4 = o.reshape(4, KS, qlim)
        numer = o4[:, 0:64, :qn1]                 # [4, 64, qn1]
        denom = o4[:, 64, :qn1].copy()            # [4, qn1]
        nadd = min(NF, qn1)
        denom[:, :nadd] += cnt[None, :nadd]
        denom += 1e-30
        res_o = (numer / denom[:, None, :]).transpose(2, 0, 1)  # [qn1, 4, 64]
        full[b, qp[:qn1], hg * HG:(hg + 1) * HG] = res_o.reshape(qn1, HG)
    return full


# revision 12
# speedup vs baseline: 2.0202x; 1.2192x over previous
"""Distributed multi-head attention kernel for 8 TRN2 NeuronCores.

Sharding: core c handles batch b = c//2 and head-group hg = c%2 (4 of 8
heads = 256 output columns).  Output slices are disjoint -> no collectives.

Device algorithm (per core), bf16 matmuls / f32 softmax:
  - host compacts BOTH axes: keys with v_mask=1 first (ascending) -> only
    NU=ceil(max_unmasked_k/128) key chunks, and queries with q_mask=1
    first (ascending) -> only NQ=ceil(max_unmasked_q/512) query tiles
    (masked queries' outputs are exactly zero -> never computed)
  - scores in S^T layout [k', q']; block (c,t) computed only if causally
    live (union over batches -> SPMD-identical graph), and narrowed to
    the live q-column range [js, 512)
  - exp via scalar ACT (bias = per-key -1e10 padding mask, scale 0.125),
    writing U bf16; straddling blocks multiply a causal 0/1 mask that is
    generated ON DEVICE via tensor_scalar(is_ge, qpos, kpos)
  - PV accumulates O^T[65*4, q'] in one PSUM tile per q-tile; row 64 of
    each head (ones column in VW) is the softmax denominator
  - dead live-queries (all causal keys masked): fvec fix matmuls as in
    the reference semantics; denominator counts added on host
  - finalize is HOST-side: device only copies PSUM->SBUF (bf16) and DMAs
    raw O^T out; host divides by the denominator row, transposes, and
    scatters into unmasked query rows
  - scalar engine queue carries ONLY the exp ACTs (the critical path);
    DMA issues go to sync HWDGE / early-scalar / gpsimd SWDGE rings
  - emission is software-pipelined (PV lags its block by one) so the ACT
    queue never waits on PSUM score slots
"""

import numpy as np
import ml_dtypes

BF = ml_dtypes.bfloat16
B, S, D = 4, 2048, 512
HG = 256          # output columns per core (4 heads x 64)
KS = 65           # head value width + ones column
NEG = np.float32(-1e10)

_CACHE = {}


def _structure(v_mask, q_mask):
    """Both-axis compaction + union block liveness (SPMD-safe)."""
    kperms, kn1s, qperms, qn1s = [], [], [], []
    for b in range(B):
        unm = np.where(v_mask[b] == 1)[0]
        msk = np.where(v_mask[b] == 0)[0]
        kperms.append(np.concatenate([unm, msk]))
        kn1s.append(len(unm))
        unq = np.where(q_mask[b] == 1)[0]
        msq = np.where(q_mask[b] == 0)[0]
        qperms.append(np.concatenate([unq, msq]))
        qn1s.append(len(unq))
    NU = int(max(-(-n // 128) for n in kn1s))
    NQ = int(max(-(-n // 512) for n in qn1s))

    blocks = []   # per t: list of (c, js, band)
    for t in range(NQ):
        bl = []
        for c in range(NU):
            live = False
            band = False
            starts = []
            per_batch = []
            for b in range(B):
                kseg = kperms[b][128 * c:min(128 * (c + 1), kn1s[b])]
                qseg = qperms[b][512 * t:min(512 * (t + 1), qn1s[b])]
                if len(kseg) == 0 or len(qseg) == 0:
                    continue
                lo, hi = int(kseg[0]), int(kseg[-1])
                if lo <= int(qseg[-1]):
                    live = True
                    jl = int(np.searchsorted(qseg, lo))
                    starts.append(jl)
                    per_batch.append((jl, hi, qseg))
                else:
                    band = True  # keys exist for b but all causally dead
            if not live:
                continue
            js = min(starts)
            for jl, hi, qseg in per_batch:
                if jl > js or hi > int(qseg[jl]):
                    band = True
            bl.append((c, int(js), bool(band)))
        assert bl and bl[0][0] == 0
        if bl[0][1] != 0:
            # widen the first block to full width so PV start=True
            # initializes every psO column (extra cols are masked by bmask)
            bl[0] = (0, 0, True)
        blocks.append(tuple(bl))

    # dead live-queries (fix): count per batch of unmasked q with all
    # causally-allowed keys masked
    nfix = 0
    for b in range(B):
        if v_mask[b, 0] == 0:
            first_one = int(np.argmax(v_mask[b] > 0))
            ndead = int(np.sum(q_mask[b, :first_one] == 1))
            nfix = max(nfix, ndead)
    NF = max(nfix, 1)  # keep graph static; zero-filled if unused

    return (kperms, kn1s, qperms, qn1s, NU, NQ, tuple(blocks), NF)


def _build(NU, NQ, blocks, NF):
    import concourse.bass as bass  # noqa: F401
    from concourse import bacc
    import concourse.mybir as mybir
    from concourse.tile import TileContext

    F32 = mybir.dt.float32
    BF16 = mybir.dt.bfloat16
    Exp = mybir.ActivationFunctionType.Exp
    GE = mybir.AluOpType.is_ge
    klim = NU * 128
    qlim = NQ * 512
    kst = [min(512, klim - 512 * i) for i in range(-(-klim // 512))]
    bands = [(c, t) for t in range(NQ) for (c, js, bd) in blocks[t] if bd]
    band_idx = {ct: i for i, ct in enumerate(bands)}
    band_js = {(c, t): js for t in range(NQ) for (c, js, bd) in blocks[t]
               if bd}

    nc = bacc.Bacc()
    VSP = 384 if klim > 384 else 128
    assert klim > 512 and klim > VSP, "tiny-NU layout not implemented"
    k0T = nc.declare_dram_parameter("k0T", [128, 4 * 512], BF16,
                                    isOutput=False)
    k1T = nc.declare_dram_parameter("k1T", [128, 4 * (klim - 512)], BF16,
                                    isOutput=False)
    q0T = nc.declare_dram_parameter("q0T", [128, 4 * 512], BF16,
                                    isOutput=False)
    if qlim > 512:
        q1T = nc.declare_dram_parameter("q1T", [128, 4 * (qlim - 512)],
                                        BF16, isOutput=False)
    v0T = nc.declare_dram_parameter("v0T", [128, 4 * VSP], BF16,
                                    isOutput=False)
    v1T = nc.declare_dram_parameter("v1T", [128, 4 * (klim - VSP)], BF16,
                                    isOutput=False)
    wq = nc.declare_dram_parameter("wq", [D, HG], BF16, isOutput=False)
    wk = nc.declare_dram_parameter("wk", [D, HG], BF16, isOutput=False)
    wv = nc.declare_dram_parameter("wv", [D, HG], BF16, isOutput=False)
    vbias = nc.declare_dram_parameter("vbias", [128, NU], F32, isOutput=False)
    kpos = nc.declare_dram_parameter("kpos", [128, NU], F32, isOutput=False)
    qpos = nc.declare_dram_parameter("qpos", [128, qlim], F32, isOutput=False)
    fvec = nc.declare_dram_parameter("fvec", [128, 4 * NF], BF16,
                                     isOutput=False)
    outT = nc.declare_dram_parameter("outT", [4 * KS, qlim], BF16,
                                     isOutput=True)

    with TileContext(nc) as tc:
        with tc.tile_pool(name="sb", bufs=1) as sb, \
             tc.tile_pool(name="ps", bufs=1, space="PSUM") as ps:

            def sbt(name, shape, dtype, bufs=1, tag=None):
                return sb.tile(shape, dtype, name=name, tag=tag or name,
                               bufs=bufs)

            kt0 = sbt("kt0", [128, 4, 512], BF16)
            kt1 = sbt("kt1", [128, 4, klim - 512], BF16)
            qt0 = sbt("qt0", [128, 4, 512], BF16)
            qt1 = (sbt("qt1", [128, 4, qlim - 512], BF16)
                   if qlim > 512 else None)
            vt0 = sbt("vt0", [128, 4, VSP], BF16)
            vt1 = sbt("vt1", [128, 4, klim - VSP], BF16)
            wk_sb = sbt("wk_sb", [128, 4, HG], BF16)
            wq_sb = sbt("wq_sb", [128, 4, HG], BF16)
            wv_sb = sbt("wv_sb", [128, 4, HG], BF16)
            vbias_sb = sbt("vbias_sb", [128, NU], F32)
            kpos_sb = sbt("kpos_sb", [128, NU], F32)
            qpos_sb = sbt("qpos_sb", [128, qlim], F32)
            fvec_sb = sbt("fvec_sb", [128, 4 * NF], BF16)
            cw = sbt("cw", [128, 16], BF16)
            kwT = [sbt(f"kwT{i}", [128, klim], BF16) for i in range(2)]
            qwT = [sbt(f"qwT{i}", [128, qlim], BF16) for i in range(2)]
            vw = [sbt(f"vw{i}", [128, 4 * KS], BF16) for i in range(NU)]
            bm = [sbt(f"bm{i}", [128, 512], BF16) for i in range(len(bands))]
            oT = [sbt(f"oT{t}", [KS, 4 * 512], BF16) for t in range(NQ)]

            def kq_mov(which, Dc, st2, w):
                if which == "k":
                    return (kt0[:, Dc, 0:w] if st2 == 0
                            else kt1[:, Dc, 512 * (st2 - 1):
                                     512 * (st2 - 1) + w])
                return (qt0[:, Dc, 0:w] if st2 == 0
                        else qt1[:, Dc, 512 * (st2 - 1):512 * (st2 - 1) + w])

            def v_mov(Dc, st):
                lo = 128 * st
                if lo < VSP:
                    return vt0[:, Dc, lo:lo + 128]
                return vt1[:, Dc, lo - VSP:lo - VSP + 128]

            # --- DMA issues: sync HWDGE + early scalar HWDGE + gpsimd SWDGE
            nc.sync.dma_start(out=wk_sb,
                              in_=wk.rearrange("(c p) o -> p c o", p=128))
            nc.sync.dma_start(out=kt0,
                              in_=k0T.rearrange("p (c s) -> p c s", c=4))
            nc.scalar.dma_start(out=qt0,
                                in_=q0T.rearrange("p (c s) -> p c s", c=4))
            nc.scalar.dma_start(out=wq_sb,
                                in_=wq.rearrange("(c p) o -> p c o", p=128))
            nc.sync.dma_start(out=wv_sb,
                              in_=wv.rearrange("(c p) o -> p c o", p=128))
            nc.sync.dma_start(out=vt0,
                              in_=v0T.rearrange("p (c s) -> p c s", c=4))
            nc.sync.dma_start(out=kt1,
                              in_=k1T.rearrange("p (c s) -> p c s", c=4))
            nc.scalar.dma_start(out=vt1,
                                in_=v1T.rearrange("p (c s) -> p c s", c=4))
            if qlim > 512:
                nc.scalar.dma_start(
                    out=qt1, in_=q1T.rearrange("p (c s) -> p c s", c=4))
            nc.gpsimd.dma_start(out=qpos_sb, in_=qpos[:])
            nc.gpsimd.dma_start(out=kpos_sb, in_=kpos[:])
            nc.gpsimd.dma_start(out=vbias_sb, in_=vbias[:])
            nc.gpsimd.dma_start(out=fvec_sb, in_=fvec[:])

            # ones columns of VW (gpsimd, off the critical engines)
            for st in range(NU):
                nc.gpsimd.memset(
                    vw[st].rearrange("p (h j) -> p h j", j=KS)[:, :, 64:65],
                    1.0)

            # PE warm-up burst: keeps the HAM activity window busy while the
            # first DMAs land so projections run at 2.4 GHz, not 1.2
            nc.vector.memset(cw, 0.125)
            pd = ps.tile([128, 1024], F32, name="pd", tag="ps", bufs=2)
            for i in range(24):
                nc.tensor.matmul(pd[0:16, 0:1], cw, cw[:, 0:1],
                                 start=True, stop=True)

            band_done = set()

            def gen_band(c, t):
                if (c, t) in band_done:
                    return
                band_done.add((c, t))
                js = band_js[(c, t)]
                nc.vector.tensor_scalar(
                    bm[band_idx[(c, t)]][:, js:512],
                    qpos_sb[:, 512 * t + js:512 * (t + 1)],
                    kpos_sb[:, c:c + 1], None, GE)

            # --- projections (JIT-scheduled below) ---
            kq_done = set()
            v_done = set()

            def proj_kq(which, dc, st2):
                if (which, dc, st2) in kq_done:
                    return
                kq_done.add((which, dc, st2))
                dst, w_sb2 = ((kwT, wk_sb) if which == "k"
                              else (qwT, wq_sb))
                w = kst[st2] if which == "k" else 512
                p = ps.tile([128, 1024], F32, name="pprj", tag="ps", bufs=2)
                for Dc in range(4):
                    nc.tensor.matmul(
                        p[:, 0:w],
                        w_sb2[:, Dc, 128 * dc:128 * (dc + 1)],
                        kq_mov(which, Dc, st2, w),
                        start=(Dc == 0), stop=(Dc == 3))
                nc.vector.tensor_copy(dst[dc][:, 512 * st2:512 * st2 + w],
                                      p[:, 0:w])

            def proj_v(st):
                if st in v_done:
                    return
                v_done.add(st)
                p = ps.tile([128, 1024], F32, name="pprjv", tag="ps", bufs=2)
                for Dc in range(4):
                    nc.tensor.matmul(p[:, 0:HG],
                                     v_mov(Dc, st),
                                     wv_sb[:, Dc, :],
                                     start=(Dc == 0), stop=(Dc == 3))
                nc.vector.tensor_copy(
                    vw[st].rearrange("p (h j) -> p h j", j=KS)[:, :, 0:64],
                    p[:, 0:HG].rearrange("p (h j) -> p h j", j=64))

            # --- attention: q-tile passes, both head-pair chains, pipelined
            for dc in range(2):
                proj_kq("q", dc, 0)
                proj_kq("k", dc, 0)

            for t in range(NQ):
                bl = blocks[t]
                nbl = len(bl)
                psO = ps.tile([KS, 4 * 512], F32, name="psO", tag="psO",
                              bufs=1)
                psO3 = psO.rearrange("p (h w) -> p h w", w=512)
                pend = []  # delayed PVs: (c, js, dc)
                for bi in range(nbl + 1):
                    if bi < nbl:
                        c, js, bd = bl[bi]
                        w = 512 - js
                        # prefetch projections for the NEXT block / q-tile
                        if bi + 1 < nbl:
                            cn = bl[bi + 1][0]
                            for dc in range(2):
                                proj_kq("k", dc, cn // 4)
                        elif t + 1 < NQ:
                            for dc in range(2):
                                proj_kq("q", dc, t + 1)
                        Us = []
                        psSs = []
                        for dc in range(2):
                            psS = ps.tile([128, 1024], F32, name="psS",
                                          tag="ps", bufs=2)
                            psS3 = psS.rearrange("p (h w) -> p h w", w=512)
                            for hp in range(2):
                                nc.tensor.matmul(
                                    psS3[:, hp, js:512],
                                    kwT[dc][64 * hp:64 * (hp + 1),
                                            128 * c:128 * (c + 1)],
                                    qwT[dc][64 * hp:64 * (hp + 1),
                                            512 * t + js:512 * (t + 1)],
                                    start=True, stop=True)
                            psSs.append(psS3)
                        for dc in range(2):
                            U = sb.tile([128, 1024], BF16, name="U", tag="U",
                                        bufs=6)
                            U3 = U.rearrange("p (h w) -> p h w", w=512)
                            nc.scalar.activation(
                                U3[:, :, js:512], psSs[dc][:, :, js:512],
                                Exp, bias=vbias_sb[:, c:c + 1], scale=0.125)
                            Us.append(U3)
                        if bd:
                            gen_band(c, t)
                            bmv = bm[band_idx[(c, t)]]
                            for dc in range(2):
                                for hp in range(2):
                                    nc.vector.tensor_mul(
                                        Us[dc][:, hp, js:512],
                                        Us[dc][:, hp, js:512],
                                        bmv[:, js:512])
                    # delayed PVs from the previous block
                    for (pc, pjs, pdc, pU3) in pend:
                        proj_v(pc)
                        for hp in range(2):
                            h = 2 * pdc + hp
                            last = (t > 0 or NF == 0) and bi == nbl
                            nc.tensor.matmul(
                                psO3[:, h, pjs:512],
                                vw[pc][:, KS * h:KS * (h + 1)],
                                pU3[:, hp, pjs:512],
                                start=(pc == 0), stop=last,
                                skip_group_check=True)
                    pend = ([(c, js, dcx, Us[dcx]) for dcx in range(2)]
                            if bi < nbl else [])
                # dead-query fix adds into q columns 0:NF at t=0
                if t == 0 and NF > 0:
                    for h in range(4):
                        for Dc in range(4):
                            nc.tensor.matmul(
                                psO3[0:64, h, 0:NF],
                                wv_sb[:, Dc, 64 * h:64 * (h + 1)],
                                fvec_sb[:, NF * Dc:NF * (Dc + 1)],
                                start=False, stop=(Dc == 3),
                                skip_group_check=True)
                nc.vector.tensor_copy(oT[t], psO)
                nc.sync.dma_start(
                    out=outT.rearrange("(h p) q -> p h q", p=KS)[
                        :, :, 512 * t:512 * (t + 1)],
                    in_=oT[t].rearrange("p (h w) -> p h w", w=512))

    nc.compile()
    return nc


def _prep_inputs(q, k, v, v_mask, q_mask, Wq, Wk, Wv, st):
    kperms, kn1s, qperms, qn1s, NU, NQ, blocks, NF = st
    klim, qlim = NU * 128, NQ * 512
    q = np.asarray(q, np.float32)
    k = np.asarray(k, np.float32)
    v = np.asarray(v, np.float32)
    Wq = np.asarray(Wq, np.float32)
    Wk = np.asarray(Wk, np.float32)
    Wv = np.asarray(Wv, np.float32)

    in_maps = []
    fin = []  # per-core host finalize info
    for core in range(8):
        b, hg = core // 2, core % 2
        cs = slice(hg * HG, (hg + 1) * HG)
        kperm, kn1 = kperms[b], kn1s[b]
        qperm, qn1 = qperms[b], qn1s[b]
        kp = kperm[:klim]
        qp = qperm[:qlim]

        ranks = np.arange(klim)
        vb = np.where(ranks < kn1, np.float32(0), NEG).astype(np.float32)
        kposv = np.where(ranks < kn1, kp, 4096).astype(np.float32)
        qposv = np.where(np.arange(qlim) < qn1, qp, 4095).astype(np.float32)

        # dead live-query fix
        fix = np.zeros((S, NF), np.float32)
        cnt = np.zeros(NF, np.float32)
        if v_mask[b, 0] == 0:
            first_one = int(np.argmax(v_mask[b] > 0))
            ks_ = np.arange(S)
            jcol = 0
            for dj in range(first_one):
                if q_mask[b, dj] != 1:
                    continue
                sel = ((ks_ <= dj) & (v_mask[b] == 0)) | \
                      ((ks_ > dj) & (v_mask[b] == 1))
                fix[:, jcol] = sel.astype(np.float32)
                cnt[jcol] = fix[:, jcol].sum()
                jcol += 1
        fv = (v[b].T @ fix).astype(np.float32)  # [512, NF]

        VSP = 384 if klim > 384 else 128

        def tiles(xT, lo, hi):
            # [512, lim] -> contiguous [128, 4*(hi-lo)] in (p, c, s) order
            t4 = xT.reshape(4, 128, -1)[:, :, lo:hi]
            return np.ascontiguousarray(
                t4.transpose(1, 0, 2).reshape(128, -1)).astype(BF)

        kTb = k[b][kp].T
        qTb = q[b][qp].T
        vTb = v[b][kp].T
        im = {
            "k0T": tiles(kTb, 0, 512),
            "k1T": tiles(kTb, 512, klim),
            "q0T": tiles(qTb, 0, 512),
            "v0T": tiles(vTb, 0, VSP),
            "v1T": tiles(vTb, VSP, klim),
            "wq": np.ascontiguousarray(Wq[:, cs]).astype(BF),
            "wk": np.ascontiguousarray(Wk[:, cs]).astype(BF),
            "wv": np.ascontiguousarray(Wv[:, cs]).astype(BF),
            "vbias": np.ascontiguousarray(vb.reshape(NU, 128).T),
            "kpos": np.ascontiguousarray(kposv.reshape(NU, 128).T),
            "qpos": np.ascontiguousarray(
                np.broadcast_to(qposv, (128, qlim))),
            "fvec": np.ascontiguousarray(
                fv.reshape(4, 128, NF).transpose(1, 0, 2)
                .reshape(128, 4 * NF)).astype(BF),
        }
        if qlim > 512:
            im["q1T"] = tiles(qTb, 512, qlim)
        in_maps.append(im)
        fin.append((b, hg, qp, qn1, cnt))
    return in_maps, fin


def kernel(q, k, v, v_mask, q_mask, Wq, Wk, Wv, _trace=False):
    from concourse.bass_utils import run_bass_kernel_spmd

    v_mask_f = np.asarray(v_mask, np.float32)
    q_mask_f = np.asarray(q_mask, np.float32)
    st = _structure(v_mask_f, q_mask_f)
    kperms, kn1s, qperms, qn1s, NU, NQ, blocks, NF = st
    key = (NU, NQ, blocks, NF)
    if _CACHE.get("key") != key:
        _CACHE["nc"] = _build(NU, NQ, blocks, NF)
        _CACHE["key"] = key
    nc = _CACHE["nc"]
    in_maps, fin = _prep_inputs(q, k, v, v_mask_f, q_mask_f, Wq, Wk, Wv, st)
    res = run_bass_kernel_spmd(nc, in_maps, core_ids=list(range(8)),
                               trace=_trace)
    _CACHE["last_result"] = res

    qlim = NQ * 512
    full = np.zeros((B, S, 2 * HG), np.float32)
    for core in range(8):
        b, hg, qp, qn1, cnt = fin[core]
        o = np.asarray(res.results[core]["outT"], np.float32)  # [260, qlim]
        o4 = o.reshape(4, KS, qlim)
        numer = o4[:, 0:64, :qn1]                 # [4, 64, qn1]
        denom = o4[:, 64, :qn1].copy()            # [4, qn1]
        nadd = min(NF, qn1)
        denom[:, :nadd] += cnt[None, :nadd]
        denom += 1e-30
        res_o = (numer / denom[:, None, :]).transpose(2, 0, 1)  # [qn1, 4, 64]
        full[b, qp[:qn1], hg * HG:(hg + 1) * HG] = res_o.reshape(qn1, HG)
    return full


# revision 24
# speedup vs baseline: 2.0325x; 1.0061x over previous
"""Distributed multi-head attention kernel for 8 TRN2 NeuronCores.

Sharding: core c handles batch b = c//2 and head-group hg = c%2 (4 of 8
heads = 256 output columns).  Output slices are disjoint -> no collectives.

Device algorithm (per core), bf16 matmuls / f32 softmax:
  - host compacts BOTH axes: keys with v_mask=1 first (ascending) -> only
    NU=ceil(max_unmasked_k/128) key chunks, and queries with q_mask=1
    first (ascending) -> only NQ=ceil(max_unmasked_q/512) query tiles
    (masked queries' outputs are exactly zero -> never computed)
  - scores in S^T layout [k', q']; block (c,t) computed only if causally
    live (union over batches -> SPMD-identical graph), and narrowed to
    the live q-column range [js, 512)
  - exp via scalar ACT (bias = per-key -1e10 padding mask, scale 0.125),
    writing U bf16; straddling blocks multiply a causal 0/1 mask that is
    generated ON DEVICE via tensor_scalar(is_ge, qpos, kpos)
  - PV accumulates O^T[65*4, q'] in one PSUM tile per q-tile; row 64 of
    each head (ones column in VW) is the softmax denominator
  - dead live-queries (all causal keys masked): fvec fix matmuls as in
    the reference semantics; denominator counts added on host
  - finalize is HOST-side: device only copies PSUM->SBUF (bf16) and DMAs
    raw O^T out; host divides by the denominator row, transposes, and
    scatters into unmasked query rows
  - scalar engine queue carries ONLY the exp ACTs (the critical path);
    DMA issues go to sync HWDGE / early-scalar / gpsimd SWDGE rings
  - emission is software-pipelined (PV lags its block by one) so the ACT
    queue never waits on PSUM score slots
"""

import numpy as np
import ml_dtypes

BF = ml_dtypes.bfloat16
B, S, D = 4, 2048, 512
HG = 256          # output columns per core (4 heads x 64)
KS = 65           # head value width + ones column
NEG = np.float32(-1e10)

_CACHE = {}


def _structure(v_mask, q_mask):
    """Both-axis compaction + union block liveness (SPMD-safe)."""
    kperms, kn1s, qperms, qn1s = [], [], [], []
    for b in range(B):
        unm = np.where(v_mask[b] == 1)[0]
        msk = np.where(v_mask[b] == 0)[0]
        kperms.append(np.concatenate([unm, msk]))
        kn1s.append(len(unm))
        unq = np.where(q_mask[b] == 1)[0]
        msq = np.where(q_mask[b] == 0)[0]
        qperms.append(np.concatenate([unq, msq]))
        qn1s.append(len(unq))
    NU = int(max(-(-n // 128) for n in kn1s))
    NQ = int(max(-(-n // 512) for n in qn1s))

    blocks = []   # per t: list of (c, js, band)
    for t in range(NQ):
        bl = []
        for c in range(NU):
            live = False
            band = False
            starts = []
            per_batch = []
            for b in range(B):
                kseg = kperms[b][128 * c:min(128 * (c + 1), kn1s[b])]
                qseg = qperms[b][512 * t:min(512 * (t + 1), qn1s[b])]
                if len(kseg) == 0 or len(qseg) == 0:
                    continue
                lo, hi = int(kseg[0]), int(kseg[-1])
                if lo <= int(qseg[-1]):
                    live = True
                    jl = int(np.searchsorted(qseg, lo))
                    starts.append(jl)
                    per_batch.append((jl, hi, qseg))
                else:
                    band = True  # keys exist for b but all causally dead
            if not live:
                continue
            js = min(starts)
            for jl, hi, qseg in per_batch:
                if jl > js or hi > int(qseg[jl]):
                    band = True
            bl.append((c, int(js), bool(band)))
        assert bl and bl[0][0] == 0
        if bl[0][1] != 0:
            # widen the first block to full width so PV start=True
            # initializes every psO column (extra cols are masked by bmask)
            bl[0] = (0, 0, True)
        blocks.append(tuple(bl))

    # dead live-queries (fix): count per batch of unmasked q with all
    # causally-allowed keys masked
    nfix = 0
    for b in range(B):
        if v_mask[b, 0] == 0:
            first_one = int(np.argmax(v_mask[b] > 0))
            ndead = int(np.sum(q_mask[b, :first_one] == 1))
            nfix = max(nfix, ndead)
    NF = max(nfix, 1)  # keep graph static; zero-filled if unused

    return (kperms, kn1s, qperms, qn1s, NU, NQ, tuple(blocks), NF)


def _build(NU, NQ, blocks, NF):
    import concourse.bass as bass  # noqa: F401
    from concourse import bacc
    import concourse.mybir as mybir
    from concourse.tile import TileContext

    F32 = mybir.dt.float32
    F16 = mybir.dt.float16
    BF16 = mybir.dt.bfloat16
    Exp = mybir.ActivationFunctionType.Exp
    GE = mybir.AluOpType.is_ge
    klim = NU * 128
    qlim = NQ * 512
    kst = [min(512, klim - 512 * i) for i in range(-(-klim // 512))]
    bands = [(c, t) for t in range(NQ) for (c, js, bd) in blocks[t] if bd]
    band_idx = {ct: i for i, ct in enumerate(bands)}
    band_js = {(c, t): js for t in range(NQ) for (c, js, bd) in blocks[t]
               if bd}

    nc = bacc.Bacc()
    VSP = 384 if klim > 384 else 128
    assert klim > 512 and klim > VSP, "tiny-NU layout not implemented"
    k0T = nc.declare_dram_parameter("k0T", [128, 4 * 512], BF16,
                                    isOutput=False)
    k1T = nc.declare_dram_parameter("k1T", [128, 4 * (klim - 512)], BF16,
                                    isOutput=False)
    q0T = nc.declare_dram_parameter("q0T", [128, 4 * 512], BF16,
                                    isOutput=False)
    if qlim > 512:
        q1T = nc.declare_dram_parameter("q1T", [128, 4 * (qlim - 512)],
                                        BF16, isOutput=False)
    v0T = nc.declare_dram_parameter("v0T", [128, 4 * VSP], BF16,
                                    isOutput=False)
    v1T = nc.declare_dram_parameter("v1T", [128, 4 * (klim - VSP)], BF16,
                                    isOutput=False)
    wq = nc.declare_dram_parameter("wq", [D, HG], BF16, isOutput=False)
    wk = nc.declare_dram_parameter("wk", [D, HG], BF16, isOutput=False)
    wv = nc.declare_dram_parameter("wv", [D, HG], BF16, isOutput=False)
    vbias = nc.declare_dram_parameter("vbias", [128, NU], F32, isOutput=False)
    kpos = nc.declare_dram_parameter("kpos", [128, NU], F32, isOutput=False)
    qpos = nc.declare_dram_parameter("qpos", [128, qlim], F32, isOutput=False)
    outT = nc.declare_dram_parameter("outT", [KS, NQ * 2048], BF16,
                                     isOutput=True)

    with TileContext(nc) as tc:
        with tc.tile_pool(name="sb", bufs=1) as sb, \
             tc.tile_pool(name="ps", bufs=1, space="PSUM") as ps:

            def sbt(name, shape, dtype, bufs=1, tag=None):
                return sb.tile(shape, dtype, name=name, tag=tag or name,
                               bufs=bufs)

            kt0 = sbt("kt0", [128, 4, 512], BF16)
            kt1 = sbt("kt1", [128, 4, klim - 512], BF16)
            qt0 = sbt("qt0", [128, 4, 512], BF16)
            qt1 = (sbt("qt1", [128, 4, qlim - 512], BF16)
                   if qlim > 512 else None)
            vt0 = sbt("vt0", [128, 4, VSP], BF16)
            vt1 = sbt("vt1", [128, 4, klim - VSP], BF16)
            wk_sb = sbt("wk_sb", [128, 4, HG], BF16)
            wq_sb = sbt("wq_sb", [128, 4, HG], BF16)
            wv_sb = sbt("wv_sb", [128, 4, HG], BF16)
            vbias_sb = sbt("vbias_sb", [128, NU], F32)
            kpos_sb = sbt("kpos_sb", [128, NU], F32)
            qpos_sb = sbt("qpos_sb", [128, qlim], F32)
            cw = sbt("cw", [128, 16], BF16)
            kwT = [sbt(f"kwT{i}", [128, klim], BF16) for i in range(2)]
            qwT = [sbt(f"qwT{i}", [128, qlim], BF16) for i in range(2)]
            vw = [sbt(f"vw{i}", [128, 4 * KS], BF16) for i in range(NU)]
            bm = [sbt(f"bm{i}", [128, 512], BF16) for i in range(len(bands))]
            oT = [sbt(f"oT{t}", [KS, 4 * 512], BF16) for t in range(NQ)]

            def kq_mov(which, Dc, st2, w):
                if which == "k":
                    return (kt0[:, Dc, 0:w] if st2 == 0
                            else kt1[:, Dc, 512 * (st2 - 1):
                                     512 * (st2 - 1) + w])
                return (qt0[:, Dc, 0:w] if st2 == 0
                        else qt1[:, Dc, 512 * (st2 - 1):512 * (st2 - 1) + w])

            def v_mov(Dc, st):
                lo = 128 * st
                if lo < VSP:
                    return vt0[:, Dc, lo:lo + 128]
                return vt1[:, Dc, lo - VSP:lo - VSP + 128]

            # --- DMA issues: sync HWDGE + early scalar HWDGE + gpsimd SWDGE
            # weights first (small, gate the projections), then the first
            # 512-column slabs, then the tails
            nc.sync.dma_start(out=wk_sb,
                              in_=wk.rearrange("(c p) o -> p c o", p=128))
            nc.scalar.dma_start(out=wq_sb,
                                in_=wq.rearrange("(c p) o -> p c o", p=128))
            nc.sync.dma_start(out=kt0,
                              in_=k0T.rearrange("p (c s) -> p c s", c=4))
            nc.scalar.dma_start(out=qt0,
                                in_=q0T.rearrange("p (c s) -> p c s", c=4))
            nc.sync.dma_start(out=wv_sb,
                              in_=wv.rearrange("(c p) o -> p c o", p=128))
            nc.sync.dma_start(out=vt0,
                              in_=v0T.rearrange("p (c s) -> p c s", c=4))
            nc.sync.dma_start(out=kt1,
                              in_=k1T.rearrange("p (c s) -> p c s", c=4))
            nc.scalar.dma_start(out=vt1,
                                in_=v1T.rearrange("p (c s) -> p c s", c=4))
            if qlim > 512:
                nc.scalar.dma_start(
                    out=qt1, in_=q1T.rearrange("p (c s) -> p c s", c=4))
            nc.gpsimd.dma_start(out=qpos_sb, in_=qpos[:])
            nc.gpsimd.dma_start(out=kpos_sb, in_=kpos[:])
            nc.gpsimd.dma_start(out=vbias_sb, in_=vbias[:])

            # ones columns of VW (gpsimd, off the critical engines)
            for st in range(NU):
                nc.gpsimd.memset(
                    vw[st].rearrange("p (h j) -> p h j", j=KS)[:, :, 64:65],
                    1.0)

            # PE warm-up burst: keeps the HAM activity window busy while the
            # first DMAs land so projections run at 2.4 GHz, not 1.2
            nc.vector.memset(cw, 0.125)
            pd = ps.tile([128, 1024], F32, name="pd", tag="ps", bufs=2)
            for i in range(30):
                nc.tensor.matmul(pd[0:16, 0:1], cw, cw[:, 0:1],
                                 start=True, stop=True)

            band_done = set()

            def gen_band(c, t):
                if (c, t) in band_done:
                    return
                band_done.add((c, t))
                js = band_js[(c, t)]
                nc.vector.tensor_scalar(
                    bm[band_idx[(c, t)]][:, js:512],
                    qpos_sb[:, 512 * t + js:512 * (t + 1)],
                    kpos_sb[:, c:c + 1], None, GE)

            # --- projections (JIT-scheduled below) ---
            kq_done = set()
            v_done = set()

            def proj_kq(which, dc, st2):
                if (which, dc, st2) in kq_done:
                    return
                kq_done.add((which, dc, st2))
                dst, w_sb2 = ((kwT, wk_sb) if which == "k"
                              else (qwT, wq_sb))
                w = kst[st2] if which == "k" else 512
                p = ps.tile([128, 1024], F32, name="pprj", tag="ps", bufs=2)
                for Dc in range(4):
                    nc.tensor.matmul(
                        p[:, 0:w],
                        w_sb2[:, Dc, 128 * dc:128 * (dc + 1)],
                        kq_mov(which, Dc, st2, w),
                        start=(Dc == 0), stop=(Dc == 3))
                nc.vector.tensor_copy(dst[dc][:, 512 * st2:512 * st2 + w],
                                      p[:, 0:w])

            def proj_v(st):
                if st in v_done:
                    return
                v_done.add(st)
                p = ps.tile([128, 1024], F32, name="pprjv", tag="ps", bufs=2)
                for Dc in range(4):
                    nc.tensor.matmul(p[:, 0:HG],
                                     v_mov(Dc, st),
                                     wv_sb[:, Dc, :],
                                     start=(Dc == 0), stop=(Dc == 3))
                nc.vector.tensor_copy(
                    vw[st].rearrange("p (h j) -> p h j", j=KS)[:, :, 0:64],
                    p[:, 0:HG].rearrange("p (h j) -> p h j", j=64))

            # --- attention: q-tile passes, both head-pair chains, pipelined
            for dc in range(2):
                proj_kq("q", dc, 0)
                proj_kq("k", dc, 0)
            v_queue = list(range(NU))

            for t in range(NQ):
                bl = blocks[t]
                nbl = len(bl)
                psO = ps.tile([KS, 4 * 512], F32, name="psO", tag="psO",
                              bufs=1)
                psO3 = psO.rearrange("p (h w) -> p h w", w=512)
                pend = []  # delayed PVs: (c, js, dc)
                for bi in range(nbl + 1):
                    if bi < nbl:
                        c, js, bd = bl[bi]
                        w = 512 - js
                        if bi + 1 < nbl:
                            cn = bl[bi + 1][0]
                            for dc in range(2):
                                proj_kq("k", dc, cn // 4)
                        Us = []
                        psSs = []
                        for dc in range(2):
                            psS = ps.tile([128, 1024], F32, name="psS",
                                          tag="ps", bufs=2)
                            psS3 = psS.rearrange("p (h w) -> p h w", w=512)
                            for hp in range(2):
                                nc.tensor.matmul(
                                    psS3[:, hp, js:512],
                                    kwT[dc][64 * hp:64 * (hp + 1),
                                            128 * c:128 * (c + 1)],
                                    qwT[dc][64 * hp:64 * (hp + 1),
                                            512 * t + js:512 * (t + 1)],
                                    start=True, stop=True)
                            psSs.append(psS3)
                        for dc in range(2):
                            U = sb.tile([128, 1024], BF16, name="U", tag="U",
                                        bufs=6)
                            U3 = U.rearrange("p (h w) -> p h w", w=512)
                            nc.scalar.activation(
                                U3[:, :, js:512], psSs[dc][:, :, js:512],
                                Exp, bias=vbias_sb[:, c:c + 1], scale=0.125)
                            Us.append(U3)
                        if bd:
                            gen_band(c, t)
                            bmv = bm[band_idx[(c, t)]]
                            for dc in range(2):
                                for hp in range(2):
                                    nc.vector.tensor_mul(
                                        Us[dc][:, hp, js:512],
                                        Us[dc][:, hp, js:512],
                                        bmv[:, js:512])
                    # delayed PVs from the previous block; spread V
                    # projections and the next-tile Q projection across the
                    # PV slots (PE slack) instead of bunching them at the
                    # q-tile boundary
                    for (pc, pjs, pdc, pU3) in pend:
                        proj_v(pc)
                        for hp in range(2):
                            h = 2 * pdc + hp
                            nc.tensor.matmul(
                                psO3[:, h, pjs:512],
                                vw[pc][:, KS * h:KS * (h + 1)],
                                pU3[:, hp, pjs:512],
                                start=(pc == 0), stop=(bi == nbl),
                                skip_group_check=True)
                    if pend:
                        if t == 0 and bi >= 1 and v_queue:
                            proj_v(v_queue.pop(0))
                        if t + 1 < NQ and bi == max(1, nbl - 4):
                            for dc in range(2):
                                proj_kq("q", dc, t + 1)
                    pend = ([(c, js, dcx, Us[dcx]) for dcx in range(2)]
                            if bi < nbl else [])
                # per-chain drain: copy + DMA chain A's half while chain B
                # finishes, then chain B's half
                for dcx in range(2):
                    nc.vector.tensor_copy(oT[t][:, 1024 * dcx:1024 * (dcx + 1)],
                                          psO[:, 1024 * dcx:1024 * (dcx + 1)])
                    nc.sync.dma_start(
                        out=outT[:, 2048 * t + 1024 * dcx:
                                 2048 * t + 1024 * (dcx + 1)],
                        in_=oT[t][:, 1024 * dcx:1024 * (dcx + 1)])

    nc.compile()
    return nc


def _prep_inputs(q, k, v, v_mask, q_mask, Wq, Wk, Wv, st):
    kperms, kn1s, qperms, qn1s, NU, NQ, blocks, NF = st
    klim, qlim = NU * 128, NQ * 512
    q = np.asarray(q, np.float32)
    k = np.asarray(k, np.float32)
    v = np.asarray(v, np.float32)
    Wq = np.asarray(Wq, np.float32)
    Wk = np.asarray(Wk, np.float32)
    Wv = np.asarray(Wv, np.float32)

    in_maps = []
    fin = []  # per-core host finalize info
    for core in range(8):
        b, hg = core // 2, core % 2
        cs = slice(hg * HG, (hg + 1) * HG)
        kperm, kn1 = kperms[b], kn1s[b]
        qperm, qn1 = qperms[b], qn1s[b]
        kp = kperm[:klim]
        qp = qperm[:qlim]

        ranks = np.arange(klim)
        vb = np.where(ranks < kn1, np.float32(0), NEG).astype(np.float32)
        kposv = np.where(ranks < kn1, kp, 4096).astype(np.float32)
        qposv = np.where(np.arange(qlim) < qn1, qp, 4095).astype(np.float32)

        # dead live-query fix
        fix = np.zeros((S, NF), np.float32)
        cnt = np.zeros(NF, np.float32)
        if v_mask[b, 0] == 0:
            first_one = int(np.argmax(v_mask[b] > 0))
            ks_ = np.arange(S)
            jcol = 0
            for dj in range(first_one):
                if q_mask[b, dj] != 1:
                    continue
                sel = ((ks_ <= dj) & (v_mask[b] == 0)) | \
                      ((ks_ > dj) & (v_mask[b] == 1))
                fix[:, jcol] = sel.astype(np.float32)
                cnt[jcol] = fix[:, jcol].sum()
                jcol += 1
        # dead-query numerators computed HOST-side: Wv^T (v^T F)  [256, NF]
        fv = (v[b].T @ fix).astype(np.float32)
        fixmat = (Wv[:, cs].T @ fv).astype(np.float32)

        VSP = 384 if klim > 384 else 128

        def tiles(xT, lo, hi):
            # [512, lim] -> contiguous [128, 4*(hi-lo)] in (p, c, s) order
            t4 = xT.reshape(4, 128, -1)[:, :, lo:hi]
            return np.ascontiguousarray(
                t4.transpose(1, 0, 2).reshape(128, -1)).astype(BF)

        kTb = k[b][kp].T
        qTb = q[b][qp].T
        vTb = v[b][kp].T
        im = {
            "k0T": tiles(kTb, 0, 512),
            "k1T": tiles(kTb, 512, klim),
            "q0T": tiles(qTb, 0, 512),
            "v0T": tiles(vTb, 0, VSP),
            "v1T": tiles(vTb, VSP, klim),
            "wq": np.ascontiguousarray(Wq[:, cs]).astype(BF),
            "wk": np.ascontiguousarray(Wk[:, cs]).astype(BF),
            "wv": np.ascontiguousarray(Wv[:, cs]).astype(BF),
            "vbias": np.ascontiguousarray(vb.reshape(NU, 128).T),
            "kpos": np.ascontiguousarray(kposv.reshape(NU, 128).T),
            "qpos": np.ascontiguousarray(
                np.broadcast_to(qposv, (128, qlim))),
        }
        if qlim > 512:
            im["q1T"] = tiles(qTb, 512, qlim)
        in_maps.append(im)
        fin.append((b, hg, qp, qn1, cnt, fixmat))
    return in_maps, fin


def kernel(q, k, v, v_mask, q_mask, Wq, Wk, Wv, _trace=False):
    from concourse.bass_utils import run_bass_kernel_spmd

    v_mask_f = np.asarray(v_mask, np.float32)
    q_mask_f = np.asarray(q_mask, np.float32)
    st = _structure(v_mask_f, q_mask_f)
    kperms, kn1s, qperms, qn1s, NU, NQ, blocks, NF = st
    key = (NU, NQ, blocks, NF)
    if _CACHE.get("key") != key:
        _CACHE["nc"] = _build(NU, NQ, blocks, NF)
        _CACHE["key"] = key
    nc = _CACHE["nc"]
    in_maps, fin = _prep_inputs(q, k, v, v_mask_f, q_mask_f, Wq, Wk, Wv, st)
    res = run_bass_kernel_spmd(nc, in_maps, core_ids=list(range(8)),
                               trace=_trace)
    _CACHE["last_result"] = res

    qlim = NQ * 512
    full = np.zeros((B, S, 2 * HG), np.float32)
    for core in range(8):
        b, hg, qp, qn1, cnt, fixmat = fin[core]
        o = np.asarray(res.results[core]["outT"], np.float32)  # [65, NQ*2048]
        o4 = o.reshape(KS, NQ, 4, 512).transpose(2, 0, 1, 3) \
              .reshape(4, KS, qlim)
        numer = o4[:, 0:64, :qn1].copy()          # [4, 64, qn1]
        denom = o4[:, 64, :qn1].copy()            # [4, qn1]
        nadd = min(NF, qn1)
        numer[:, :, :nadd] += fixmat.reshape(4, 64, NF)[:, :, :nadd]
        denom[:, :nadd] += cnt[None, :nadd]
        denom += 1e-30
        res_o = (numer / denom[:, None, :]).transpose(2, 0, 1)  # [qn1, 4, 64]
        full[b, qp[:qn1], hg * HG:(hg + 1) * HG] = res_o.reshape(qn1, HG)
    return full


# revision 36
# speedup vs baseline: 2.1018x; 1.0341x over previous
"""Distributed multi-head attention kernel for 8 TRN2 NeuronCores.

Sharding: core c handles batch b = c//2 and head-group hg = c%2 (4 of 8
heads = 256 output columns).  Output slices are disjoint -> no collectives.

Device algorithm (per core), bf16 matmuls / f32 softmax:
  - host compacts BOTH axes: keys with v_mask=1 first (ascending) -> only
    NU=ceil(max_unmasked_k/128) key chunks, and queries with q_mask=1
    first (ascending) -> only NQ=ceil(max_unmasked_q/512) query tiles
    (masked queries' outputs are exactly zero -> never computed)
  - scores in S^T layout [k', q']; block (c,t) computed only if causally
    live (union over batches -> SPMD-identical graph), and narrowed to
    the live q-column range [js, 512)
  - exp via scalar ACT (bias = per-key -1e10 padding mask, scale 0.125),
    writing U bf16; straddling blocks multiply a causal 0/1 mask that is
    generated ON DEVICE via tensor_scalar(is_ge, qpos, kpos)
  - PV accumulates O^T[65*4, q'] in one PSUM tile per q-tile; row 64 of
    each head (ones column in VW) is the softmax denominator
  - dead live-queries (all causal keys masked): fvec fix matmuls as in
    the reference semantics; denominator counts added on host
  - finalize is HOST-side: device only copies PSUM->SBUF (bf16) and DMAs
    raw O^T out; host divides by the denominator row, transposes, and
    scatters into unmasked query rows
  - scalar engine queue carries ONLY the exp ACTs (the critical path);
    DMA issues go to sync HWDGE / early-scalar / gpsimd SWDGE rings
  - emission is software-pipelined (PV lags its block by one) so the ACT
    queue never waits on PSUM score slots
"""

import numpy as np
import ml_dtypes

BF = ml_dtypes.bfloat16
B, S, D = 4, 2048, 512
HG = 256          # output columns per core (4 heads x 64)
KS = 65           # head value width + ones column
NEG = np.float32(-1e10)

_CACHE = {}


def _structure(v_mask, q_mask):
    """Both-axis compaction + union block liveness (SPMD-safe)."""
    kperms, kn1s, qperms, qn1s = [], [], [], []
    for b in range(B):
        unm = np.where(v_mask[b] == 1)[0]
        msk = np.where(v_mask[b] == 0)[0]
        kperms.append(np.concatenate([unm, msk]))
        kn1s.append(len(unm))
        unq = np.where(q_mask[b] == 1)[0]
        msq = np.where(q_mask[b] == 0)[0]
        qperms.append(np.concatenate([unq, msq]))
        qn1s.append(len(unq))
    NU = int(max(-(-n // 128) for n in kn1s))
    NQ = int(max(-(-n // 512) for n in qn1s))

    blocks = []   # per t: list of (c, js, band)
    for t in range(NQ):
        bl = []
        for c in range(NU):
            live = False
            band = False
            starts = []
            per_batch = []
            for b in range(B):
                kseg = kperms[b][128 * c:min(128 * (c + 1), kn1s[b])]
                qseg = qperms[b][512 * t:min(512 * (t + 1), qn1s[b])]
                if len(kseg) == 0 or len(qseg) == 0:
                    continue
                lo, hi = int(kseg[0]), int(kseg[-1])
                if lo <= int(qseg[-1]):
                    live = True
                    jl = int(np.searchsorted(qseg, lo))
                    starts.append(jl)
                    per_batch.append((jl, hi, qseg))
                else:
                    band = True  # keys exist for b but all causally dead
            if not live:
                continue
            js = min(starts)
            for jl, hi, qseg in per_batch:
                if jl > js or hi > int(qseg[jl]):
                    band = True
            bl.append((c, int(js), bool(band)))
        assert bl and bl[0][0] == 0
        if bl[0][1] != 0:
            # widen the first block to full width so PV start=True
            # initializes every psO column (extra cols are masked by bmask)
            bl[0] = (0, 0, True)
        blocks.append(tuple(bl))

    # dead live-queries (fix): count per batch of unmasked q with all
    # causally-allowed keys masked
    nfix = 0
    for b in range(B):
        if v_mask[b, 0] == 0:
            first_one = int(np.argmax(v_mask[b] > 0))
            ndead = int(np.sum(q_mask[b, :first_one] == 1))
            nfix = max(nfix, ndead)
    NF = max(nfix, 1)  # keep graph static; zero-filled if unused

    return (kperms, kn1s, qperms, qn1s, NU, NQ, tuple(blocks), NF)


def _build(NU, NQ, blocks, NF):
    import concourse.bass as bass  # noqa: F401
    from concourse import bacc
    import concourse.mybir as mybir
    from concourse.tile import TileContext

    F32 = mybir.dt.float32
    F16 = mybir.dt.float16
    BF16 = mybir.dt.bfloat16
    Exp = mybir.ActivationFunctionType.Exp
    GE = mybir.AluOpType.is_ge
    klim = NU * 128
    qlim = NQ * 512
    kst = [min(512, klim - 512 * i) for i in range(-(-klim // 512))]
    bands = [(c, t) for t in range(NQ) for (c, js, bd) in blocks[t] if bd]
    band_idx = {ct: i for i, ct in enumerate(bands)}
    band_js = {(c, t): js for t in range(NQ) for (c, js, bd) in blocks[t]
               if bd}

    nc = bacc.Bacc()
    VSP = 384 if klim > 384 else 128
    assert klim > 512 and klim > VSP, "tiny-NU layout not implemented"
    k0T = nc.declare_dram_parameter("k0T", [128, 4 * 512], BF16,
                                    isOutput=False)
    k1T = nc.declare_dram_parameter("k1T", [128, 4 * (klim - 512)], BF16,
                                    isOutput=False)
    q0T = nc.declare_dram_parameter("q0T", [128, 4 * 512], BF16,
                                    isOutput=False)
    if qlim > 512:
        q1T = nc.declare_dram_parameter("q1T", [128, 4 * (qlim - 512)],
                                        BF16, isOutput=False)
    v0T = nc.declare_dram_parameter("v0T", [128, 4 * VSP], BF16,
                                    isOutput=False)
    v1T = nc.declare_dram_parameter("v1T", [128, 4 * (klim - VSP)], BF16,
                                    isOutput=False)
    wq = nc.declare_dram_parameter("wq", [D, HG], BF16, isOutput=False)
    wk = nc.declare_dram_parameter("wk", [D, HG], BF16, isOutput=False)
    wv = nc.declare_dram_parameter("wv", [D, HG], BF16, isOutput=False)
    vbias = nc.declare_dram_parameter("vbias", [128, NU], F32, isOutput=False)
    jthr = nc.declare_dram_parameter("jthr", [128, NU * NQ], F32,
                                     isOutput=False)
    outT = nc.declare_dram_parameter("outT", [KS, NQ * 2048], BF16,
                                     isOutput=True)

    with TileContext(nc) as tc:
        with tc.tile_pool(name="sb", bufs=1) as sb, \
             tc.tile_pool(name="ps", bufs=1, space="PSUM") as ps:

            def sbt(name, shape, dtype, bufs=1, tag=None):
                return sb.tile(shape, dtype, name=name, tag=tag or name,
                               bufs=bufs)

            kt0 = sbt("kt0", [128, 4, 512], BF16)
            kt1 = sbt("kt1", [128, 4, klim - 512], BF16)
            qt0 = sbt("qt0", [128, 4, 512], BF16)
            qt1 = (sbt("qt1", [128, 4, qlim - 512], BF16)
                   if qlim > 512 else None)
            vt0 = sbt("vt0", [128, 4, VSP], BF16)
            vt1 = sbt("vt1", [128, 4, klim - VSP], BF16)
            wk_sb = sbt("wk_sb", [128, 4, HG], BF16)
            wq_sb = sbt("wq_sb", [128, 4, HG], BF16)
            wv_sb = sbt("wv_sb", [128, 4, HG], BF16)
            vbias_sb = sbt("vbias_sb", [128, NU], F32)
            jthr_sb = sbt("jthr_sb", [128, NU * NQ], F32)
            iota_i = sbt("iota_i", [128, 512], mybir.dt.int32)
            iota_f = sbt("iota_f", [128, 512], F32)
            cw = sbt("cw", [128, 16], BF16)
            kwT = [sbt(f"kwT{i}", [128, klim], BF16) for i in range(2)]
            qwT = [sbt(f"qwT{i}", [128, qlim], BF16) for i in range(2)]
            vw = [sbt(f"vw{i}", [128, 4 * KS], BF16) for i in range(NU)]
            bm = [sbt(f"bm{i}", [128, 512], BF16) for i in range(len(bands))]
            oT = [sbt(f"oT{t}", [KS, 4 * 512], BF16) for t in range(NQ)]

            def kq_mov(which, Dc, st2, w):
                if which == "k":
                    return (kt0[:, Dc, 0:w] if st2 == 0
                            else kt1[:, Dc, 512 * (st2 - 1):
                                     512 * (st2 - 1) + w])
                return (qt0[:, Dc, 0:w] if st2 == 0
                        else qt1[:, Dc, 512 * (st2 - 1):512 * (st2 - 1) + w])

            def v_mov(Dc, st):
                lo = 128 * st
                if lo < VSP:
                    return vt0[:, Dc, lo:lo + 128]
                return vt1[:, Dc, lo - VSP:lo - VSP + 128]

            # --- DMA issues: sync HWDGE + early scalar HWDGE + gpsimd SWDGE
            # weights first (small, gate the projections), then the first
            # 512-column slabs, then the tails
            nc.sync.dma_start(out=wk_sb,
                              in_=wk.rearrange("(c p) o -> p c o", p=128))
            nc.scalar.dma_start(out=wq_sb,
                                in_=wq.rearrange("(c p) o -> p c o", p=128))
            nc.sync.dma_start(out=kt0,
                              in_=k0T.rearrange("p (c s) -> p c s", c=4))
            nc.scalar.dma_start(out=qt0,
                                in_=q0T.rearrange("p (c s) -> p c s", c=4))
            nc.sync.dma_start(out=wv_sb,
                              in_=wv.rearrange("(c p) o -> p c o", p=128))
            nc.sync.dma_start(out=vt0,
                              in_=v0T.rearrange("p (c s) -> p c s", c=4))
            nc.sync.dma_start(out=kt1,
                              in_=k1T.rearrange("p (c s) -> p c s", c=4))
            nc.scalar.dma_start(out=vt1,
                                in_=v1T.rearrange("p (c s) -> p c s", c=4))
            if qlim > 512:
                nc.scalar.dma_start(
                    out=qt1, in_=q1T.rearrange("p (c s) -> p c s", c=4))
            nc.gpsimd.dma_start(out=jthr_sb, in_=jthr[:])
            nc.gpsimd.dma_start(out=vbias_sb, in_=vbias[:])
            nc.gpsimd.iota(iota_i, [[1, 512]], base=0, channel_multiplier=0)
            nc.gpsimd.tensor_copy(iota_f, iota_i)

            # ones columns of VW (gpsimd, off the critical engines)
            for st in range(NU):
                nc.gpsimd.memset(
                    vw[st].rearrange("p (h j) -> p h j", j=KS)[:, :, 64:65],
                    1.0)

            # PE warm-up burst: keeps the HAM activity window busy while the
            # first DMAs land so projections run at 2.4 GHz, not 1.2
            nc.vector.memset(cw, 0.125)
            pd = ps.tile([128, 1024], F32, name="pd", tag="ps", bufs=2)
            for i in range(30):
                nc.tensor.matmul(pd[0:16, 0:1], cw, cw[:, 0:1],
                                 start=True, stop=True)

            band_done = set()

            def gen_band(c, t):
                if (c, t) in band_done:
                    return
                band_done.add((c, t))
                js = band_js[(c, t)]
                nc.vector.tensor_scalar(
                    bm[band_idx[(c, t)]][:, js:512],
                    iota_f[:, js:512],
                    jthr_sb[:, t * NU + c:t * NU + c + 1], None, GE)

            # --- projections (JIT-scheduled below) ---
            kq_done = set()
            v_done = set()

            def proj_kq(which, dc, st2):
                if (which, dc, st2) in kq_done:
                    return
                kq_done.add((which, dc, st2))
                dst, w_sb2 = ((kwT, wk_sb) if which == "k"
                              else (qwT, wq_sb))
                w = kst[st2] if which == "k" else 512
                p = ps.tile([128, 1024], F32, name="pprj", tag="ps", bufs=2)
                for Dc in range(4):
                    nc.tensor.matmul(
                        p[:, 0:w],
                        w_sb2[:, Dc, 128 * dc:128 * (dc + 1)],
                        kq_mov(which, Dc, st2, w),
                        start=(Dc == 0), stop=(Dc == 3))
                nc.vector.tensor_copy(dst[dc][:, 512 * st2:512 * st2 + w],
                                      p[:, 0:w])

            def proj_v(st):
                if st in v_done:
                    return
                v_done.add(st)
                p = ps.tile([128, 1024], F32, name="pprjv", tag="ps", bufs=2)
                for Dc in range(4):
                    nc.tensor.matmul(p[:, 0:HG],
                                     v_mov(Dc, st),
                                     wv_sb[:, Dc, :],
                                     start=(Dc == 0), stop=(Dc == 3))
                nc.vector.tensor_copy(
                    vw[st].rearrange("p (h j) -> p h j", j=KS)[:, :, 0:64],
                    p[:, 0:HG].rearrange("p (h j) -> p h j", j=64))

            # --- attention: q-tile passes, both head-pair chains, pipelined
            for dc in range(2):
                proj_kq("q", dc, 0)
                proj_kq("k", dc, 0)
            v_queue = list(range(NU))

            for t in range(NQ):
                bl = blocks[t]
                nbl = len(bl)
                psO = ps.tile([KS, 4 * 512], F32, name="psO", tag="psO",
                              bufs=1)
                psO3 = psO.rearrange("p (h w) -> p h w", w=512)
                pend = []  # delayed PVs: (c, js, dc)
                for bi in range(nbl + 1):
                    if bi < nbl:
                        c, js, bd = bl[bi]
                        w = 512 - js
                        Us = []
                        psSs = []
                        for dc in range(2):
                            psS = ps.tile([128, 1024], F32, name="psS",
                                          tag="ps", bufs=2)
                            psS3 = psS.rearrange("p (h w) -> p h w", w=512)
                            for hp in range(2):
                                nc.tensor.matmul(
                                    psS3[:, hp, js:512],
                                    kwT[dc][64 * hp:64 * (hp + 1),
                                            128 * c:128 * (c + 1)],
                                    qwT[dc][64 * hp:64 * (hp + 1),
                                            512 * t + js:512 * (t + 1)],
                                    start=True, stop=True)
                            psSs.append(psS3)
                        for dc in range(2):
                            U = sb.tile([128, 1024], BF16, name="U", tag="U",
                                        bufs=6)
                            U3 = U.rearrange("p (h w) -> p h w", w=512)
                            nc.scalar.activation(
                                U3[:, :, js:512], psSs[dc][:, :, js:512],
                                Exp, bias=vbias_sb[:, c:c + 1], scale=0.125)
                            Us.append(U3)
                        # K-projection prefetch for the next block, AFTER
                        # this block's ACTs so it runs in their shadow
                        if bi + 1 < nbl:
                            cn = bl[bi + 1][0]
                            for dc in range(2):
                                proj_kq("k", dc, cn // 4)
                        if bd:
                            gen_band(c, t)
                            bmv = bm[band_idx[(c, t)]]
                            for dc in range(2):
                                for hp in range(2):
                                    nc.vector.tensor_mul(
                                        Us[dc][:, hp, js:512],
                                        Us[dc][:, hp, js:512],
                                        bmv[:, js:512])
                    # delayed PVs from the previous block; spread V
                    # projections and the next-tile Q projection across the
                    # PV slots (PE slack) instead of bunching them at the
                    # q-tile boundary
                    for (pc, pjs, pdc, pU3) in pend:
                        proj_v(pc)
                        for hp in range(2):
                            h = 2 * pdc + hp
                            nc.tensor.matmul(
                                psO3[:, h, pjs:512],
                                vw[pc][:, KS * h:KS * (h + 1)],
                                pU3[:, hp, pjs:512],
                                start=(pc == 0), stop=(bi == nbl),
                                skip_group_check=True)
                    if pend:
                        if bi >= 1 and v_queue:
                            proj_v(v_queue.pop(0))
                        if t + 1 < NQ:
                            dcq = bi - max(1, nbl - 4)
                            if 0 <= dcq <= 1:
                                proj_kq("q", dcq, t + 1)
                    pend = ([(c, js, dcx, Us[dcx]) for dcx in range(2)]
                            if bi < nbl else [])
                # per-chain drain: copy + DMA chain A's half while chain B
                # finishes, then chain B's half
                for dcx in range(2):
                    nc.vector.tensor_copy(oT[t][:, 1024 * dcx:1024 * (dcx + 1)],
                                          psO[:, 1024 * dcx:1024 * (dcx + 1)])
                    nc.sync.dma_start(
                        out=outT[:, 2048 * t + 1024 * dcx:
                                 2048 * t + 1024 * (dcx + 1)],
                        in_=oT[t][:, 1024 * dcx:1024 * (dcx + 1)])

    nc.compile()
    return nc


def _prep_inputs(q, k, v, v_mask, q_mask, Wq, Wk, Wv, st):
    kperms, kn1s, qperms, qn1s, NU, NQ, blocks, NF = st
    klim, qlim = NU * 128, NQ * 512
    q = np.asarray(q, np.float32)
    k = np.asarray(k, np.float32)
    v = np.asarray(v, np.float32)
    Wq = np.asarray(Wq, np.float32)
    Wk = np.asarray(Wk, np.float32)
    Wv = np.asarray(Wv, np.float32)

    in_maps = []
    fin = []  # per-core host finalize info
    for core in range(8):
        b, hg = core // 2, core % 2
        cs = slice(hg * HG, (hg + 1) * HG)
        kperm, kn1 = kperms[b], kn1s[b]
        qperm, qn1 = qperms[b], qn1s[b]
        kp = kperm[:klim]
        qp = qperm[:qlim]

        ranks = np.arange(klim)
        vb = np.where(ranks < kn1, np.float32(0), NEG).astype(np.float32)
        kposv = np.where(ranks < kn1, kp, 4096).astype(np.int64)
        qposv = np.where(np.arange(qlim) < qn1, qp, 4095).astype(np.int64)
        # per (t, c): threshold column index: bmask[p, j] = (j >= jthr)
        jt = np.zeros((128, NU * NQ), np.float32)
        for t in range(NQ):
            qseg = qposv[512 * t:512 * (t + 1)]
            for c in range(NU):
                jt[:, t * NU + c] = np.searchsorted(
                    qseg, kposv.reshape(NU, 128).T[:, c])

        # dead live-query fix
        fix = np.zeros((S, NF), np.float32)
        cnt = np.zeros(NF, np.float32)
        if v_mask[b, 0] == 0:
            first_one = int(np.argmax(v_mask[b] > 0))
            ks_ = np.arange(S)
            jcol = 0
            for dj in range(first_one):
                if q_mask[b, dj] != 1:
                    continue
                sel = ((ks_ <= dj) & (v_mask[b] == 0)) | \
                      ((ks_ > dj) & (v_mask[b] == 1))
                fix[:, jcol] = sel.astype(np.float32)
                cnt[jcol] = fix[:, jcol].sum()
                jcol += 1
        # dead-query numerators computed HOST-side: Wv^T (v^T F)  [256, NF]
        fv = (v[b].T @ fix).astype(np.float32)
        fixmat = (Wv[:, cs].T @ fv).astype(np.float32)

        VSP = 384 if klim > 384 else 128

        def tiles(xT, lo, hi):
            # [512, lim] -> contiguous [128, 4*(hi-lo)] in (p, c, s) order
            t4 = xT.reshape(4, 128, -1)[:, :, lo:hi]
            return np.ascontiguousarray(
                t4.transpose(1, 0, 2).reshape(128, -1)).astype(BF)

        kTb = k[b][kp].T
        qTb = q[b][qp].T
        vTb = v[b][kp].T
        im = {
            "k0T": tiles(kTb, 0, 512),
            "k1T": tiles(kTb, 512, klim),
            "q0T": tiles(qTb, 0, 512),
            "v0T": tiles(vTb, 0, VSP),
            "v1T": tiles(vTb, VSP, klim),
            "wq": np.ascontiguousarray(Wq[:, cs]).astype(BF),
            "wk": np.ascontiguousarray(Wk[:, cs]).astype(BF),
            "wv": np.ascontiguousarray(Wv[:, cs]).astype(BF),
            "vbias": np.ascontiguousarray(vb.reshape(NU, 128).T),
            "jthr": np.ascontiguousarray(jt),
        }
        if qlim > 512:
            im["q1T"] = tiles(qTb, 512, qlim)
        in_maps.append(im)
        fin.append((b, hg, qp, qn1, cnt, fixmat))
    return in_maps, fin


def kernel(q, k, v, v_mask, q_mask, Wq, Wk, Wv, _trace=False):
    from concourse.bass_utils import run_bass_kernel_spmd

    v_mask_f = np.asarray(v_mask, np.float32)
    q_mask_f = np.asarray(q_mask, np.float32)
    st = _structure(v_mask_f, q_mask_f)
    kperms, kn1s, qperms, qn1s, NU, NQ, blocks, NF = st
    key = (NU, NQ, blocks, NF)
    if _CACHE.get("key") != key:
        _CACHE["nc"] = _build(NU, NQ, blocks, NF)
        _CACHE["key"] = key
    nc = _CACHE["nc"]
    in_maps, fin = _prep_inputs(q, k, v, v_mask_f, q_mask_f, Wq, Wk, Wv, st)
    res = run_bass_kernel_spmd(nc, in_maps, core_ids=list(range(8)),
                               trace=_trace)
    _CACHE["last_result"] = res

    qlim = NQ * 512
    full = np.zeros((B, S, 2 * HG), np.float32)
    for core in range(8):
        b, hg, qp, qn1, cnt, fixmat = fin[core]
        o = np.asarray(res.results[core]["outT"], np.float32)  # [65, NQ*2048]
        o4 = o.reshape(KS, NQ, 4, 512).transpose(2, 0, 1, 3) \
              .reshape(4, KS, qlim)
        numer = o4[:, 0:64, :qn1].copy()          # [4, 64, qn1]
        denom = o4[:, 64, :qn1].copy()            # [4, qn1]
        nadd = min(NF, qn1)
        numer[:, :, :nadd] += fixmat.reshape(4, 64, NF)[:, :, :nadd]
        denom[:, :nadd] += cnt[None, :nadd]
        denom += 1e-30
        res_o = (numer / denom[:, None, :]).transpose(2, 0, 1)  # [qn1, 4, 64]
        full[b, qp[:qn1], hg * HG:(hg + 1) * HG] = res_o.reshape(qn1, HG)
    return full
